# revision 1
# baseline (speedup 1.0000x reference)
"""Trainium2 Bass kernel for nn_CodecTransformerLayer (sparse window attention
+ GQA + ALiBi + SwiGLU FFN), 8-core data-parallel with forward-halo recompute.

Sharding: batch(2) x seq-block(4) = 8 shards, one per core. Each core computes
its own 512 tokens end-to-end; attention needs K/V for the next 512 tokens
(window is forward-looking: dist = j - i in [0, 512]), which the core
recomputes from a 512-token halo of x instead of communicating.

Layout: feature-on-partition ("transposed") activations everywhere. All
weights and x are pre-transposed/pre-tiled on the host into the exact SBUF
layouts, so every DMA is contiguous. LayerNorm partition-dim reductions use
all-ones matmuls (gives the mean replicated across partitions for free).
ALiBi + band mask enter the score matmul as two extra contraction rows
(rank-2 decomposition of the in-band mask); out-of-band positions are zeroed
on the exp output with affine_select; invalid halo keys (last block of each
batch) get a +1e9 key-index so their logit is ~-1e9.

Matmul dtypes: bf16 for projections/FFN (error is scaled by 1e-5 residual
scales), float32r (reduced-mantissa fp32, full PE rate at N>=256) for
attention scores/AV and LN statistics. Residual path stays exact fp32.
"""

import math

import numpy as np
import ml_dtypes

import concourse.bass as bass
import concourse.mybir as mybir
import concourse.tile as tile
from concourse import bacc
from concourse.bass_utils import run_bass_kernel_spmd

P = 128
DIM = 1024
N_HEADS = 16
N_KV = 8
HD = 64
HIDDEN = 4096
WINDOW = 512
NORM_EPS = 1e-5
QK_EPS = 1e-6
B = 2
S = 2048
T_OWN = 512          # tokens owned per core
T_HALO = 1024        # own + forward halo
DS = DIM // P        # 8 d-subtiles
KS = DIM // P        # 8 hd-subtiles for wo contraction
HS = HIDDEN // P     # 32 hidden subtiles

F32 = mybir.dt.float32
F32R = mybir.dt.float32r
BF16 = mybir.dt.bfloat16
AF = mybir.ActivationFunctionType
OP = mybir.AluOpType


def _alibi_slopes(n):
    start = 2.0 ** (-(2.0 ** (-(math.log2(n) - 3))))
    return [start * start ** i for i in range(n)]


SLOPES = _alibi_slopes(N_HEADS)


# ---------------------------------------------------------------------------
# device kernel
# ---------------------------------------------------------------------------

def _build_nc():
    nc = bacc.Bacc("TRN2")

    ins = {}
    ins["xT"] = nc.dram_tensor("xT", [P, DS, T_HALO], F32, kind="ExternalInput")
    ins["wq"] = nc.dram_tensor("wq", [8, P, DS, 128], BF16, kind="ExternalInput")
    ins["wk"] = nc.dram_tensor("wk", [4, P, DS, 128], BF16, kind="ExternalInput")
    ins["wv"] = nc.dram_tensor("wv", [P, DS, 512], BF16, kind="ExternalInput")
    ins["wo"] = nc.dram_tensor("wo", [P, KS, DIM], BF16, kind="ExternalInput")
    ins["w1"] = nc.dram_tensor("w1", [HS, P, DS, 128], BF16, kind="ExternalInput")
    ins["w3"] = nc.dram_tensor("w3", [HS, P, DS, 128], BF16, kind="ExternalInput")
    ins["w2"] = nc.dram_tensor("w2", [DS, P, HS, 128], BF16, kind="ExternalInput")
    # qnw*knw folded, head-local layout [64(pad128), head]
    ins["qkw"] = nc.dram_tensor("qkw", [P, N_HEADS], F32, kind="ExternalInput")
    ins["asc"] = nc.dram_tensor("asc", [P, DS], F32, kind="ExternalInput")
    ins["fsc"] = nc.dram_tensor("fsc", [P, DS], F32, kind="ExternalInput")
    ins["kal"] = nc.dram_tensor("kal", [2, T_HALO], F32R, kind="ExternalInput")
    ins["qal"] = nc.dram_tensor("qal", [2, N_HEADS, T_OWN], F32R, kind="ExternalInput")

    out = nc.dram_tensor("out", [P, DS, T_OWN], F32, kind="ExternalOutput")

    with tile.TileContext(nc) as tc:
        _emit(nc, tc, ins, out)
    nc.finalize()
    return nc


def _ln_coeffs(nc, pool, psm, pss, inv_n, eps_ap):
    """From sum/sumsq psums (replicated across partitions), produce
    a = rstd and b = mean * rstd, both [128, 512] f32 replicated."""
    m_ = pool.tile([P, 512], F32, tag="ln_m")
    nc.vector.tensor_scalar_mul(m_[:], psm[:], inv_n)
    v_ = pool.tile([P, 512], F32, tag="ln_v")
    nc.vector.tensor_scalar_mul(v_[:], pss[:], inv_n)
    mm_ = pool.tile([P, 512], F32, tag="ln_mm")
    nc.vector.tensor_tensor(mm_[:], m_[:], m_[:], OP.mult)
    nc.vector.tensor_tensor(v_[:], v_[:], mm_[:], OP.subtract)
    s_ = pool.tile([P, 512], F32, tag="ln_s")
    nc.scalar.activation(s_[:], v_[:], AF.Sqrt, bias=eps_ap)
    nc.vector.reciprocal(s_[:], s_[:])
    b_ = pool.tile([P, 512], F32, tag="ln_b")
    nc.vector.tensor_tensor(b_[:], m_[:], s_[:], OP.mult)
    return s_, b_


def _emit(nc, tc, ins, out):
    frees = []  # keep single-tile pool handles alive; release LIFO at end

    def tile_single(shape, dtype, name):
        t, f = tc.tile(shape, dtype, name=name)
        frees.append(f)
        return t

    xT, wq, wk, wv, wo = ins["xT"], ins["wq"], ins["wk"], ins["wv"], ins["wo"]
    w1, w3, w2 = ins["w1"], ins["w3"], ins["w2"]
    qkw, asc, fsc = ins["qkw"], ins["asc"], ins["fsc"]
    kal, qal = ins["kal"], ins["qal"]

    # --- constants (kept for the whole kernel) -----------------------------
    ones_f = tile_single([P, P], F32, name="ones_f")
    nc.vector.memset(ones_f[:], 1.0)
    ones128 = tile_single([P, P], F32R, name="ones128")
    nc.vector.tensor_copy(ones128[:], ones_f[:])
    ones1 = tile_single([1, HD], F32R, name="ones1")
    nc.vector.tensor_copy(ones1[:], ones_f[0:1, 0:HD])
    qkw_sb = tile_single([P, N_HEADS], F32, name="qkw_sb")
    nc.sync.dma_start(qkw_sb[:], qkw[:])
    asc_sb = tile_single([P, DS], F32, name="asc_sb")
    nc.sync.dma_start(asc_sb[:], asc[:])
    fsc_sb = tile_single([P, DS], F32, name="fsc_sb")
    nc.sync.dma_start(fsc_sb[:], fsc[:])
    eps_n = tile_single([P, 1], F32, name="eps_n")
    nc.vector.memset(eps_n[:], NORM_EPS)
    eps_qk = tile_single([P, 1], F32, name="eps_qk")
    nc.vector.memset(eps_qk[:], QK_EPS)

    xTo = tile_single([P, DS, T_OWN], F32, name="xTo")
    nc.sync.dma_start(xTo[:], xT[:, :, 0:T_OWN])
    aoT = tile_single([P, KS, T_OWN], BF16, name="aoT")
    x2T = tile_single([P, DS, T_OWN], F32, name="x2T")

    woc = tile_single([P, KS, DIM], BF16, name="woc")
    nc.sync.dma_start(woc[:], wo[:])

    NQ = 256

    hT, free_hT = tc.tile([P, DS, T_HALO], BF16, name="hT")

    # ======================================================================
    # Phase 1: attn LN over halo tokens -> hT (bf16)
    # (attn_norm_w is folded into wq/wk/wv on the host)
    # ======================================================================
    xTh, free_xTh = tc.tile([P, DS, T_OWN], F32, name="xTh")
    nc.sync.dma_start(xTh[:], xT[:, :, T_OWN:T_HALO])
    with tc.tile_pool(name="p1c", bufs=3) as p1c, \
         tc.tile_pool(name="p1s", bufs=1) as p1s, \
         tc.tile_pool(name="psA1", bufs=2, space="PSUM") as psA1:
        for tci, xsrc in ((0, xTo), (1, xTh)):
            psm = psA1.tile([P, 512], F32, tag="st_mean")
            pss = psA1.tile([P, 512], F32, tag="st_sq")
            for ds in range(DS):
                xr = p1c.tile([P, 512], F32R, tag="xr")
                nc.vector.tensor_copy(xr[:], xsrc[:, ds])
                nc.tensor.matmul(psm[:], ones128[:], xr[:],
                                 start=(ds == 0), stop=(ds == DS - 1))
            for ds in range(DS):
                xq = p1c.tile([P, 512], F32R, tag="xq")
                nc.scalar.activation(xq[:], xsrc[:, ds], AF.Square)
                nc.tensor.matmul(pss[:], ones128[:], xq[:],
                                 start=(ds == 0), stop=(ds == DS - 1))
            s_, b_ = _ln_coeffs(nc, p1s, psm, pss, 1.0 / DIM, eps_n[:])
            for ds in range(DS):
                t_ = p1c.tile([P, 512], F32, tag="t")
                nc.vector.tensor_tensor(t_[:], xsrc[:, ds], s_[:], OP.mult)
                nc.vector.tensor_tensor(
                    hT[:, ds, tci * 512:(tci + 1) * 512], t_[:], b_[:],
                    OP.subtract)
    free_xTh()

    # ======================================================================
    # Phase 2: q/k/v projections + q/k LN (in-place) -> qext, kext, vext
    # qext[h]: rows 0..63 = q_ln (head h), row 64 = -8*slope, row 65 =
    # 8*slope*qidx. kext[g]: rows 0..63 = k_ln, row 64 = kidx, row 65 = 1.
    # vext: [tok_p, tok_sub, kv*(HD+1)] with a ones column per kv head.
    # ======================================================================
    qext, free_qext = tc.tile([P, N_HEADS, T_OWN], F32R, name="qext")
    kext, free_kext = tc.tile([P, N_KV, T_HALO], F32R, name="kext")
    vext, free_vext = tc.tile([P, DS, N_KV * (HD + 1)], F32R, name="vext")

    with tc.tile_pool(name="p2w", bufs=3) as p2w, \
         tc.tile_pool(name="p2c", bufs=2) as p2c, \
         tc.tile_pool(name="p2s", bufs=1) as p2s, \
         tc.tile_pool(name="psA2", bufs=1, space="PSUM") as psA2, \
         tc.tile_pool(name="psA2p", bufs=2, space="PSUM") as psA2p:

        # ---- q projection + interleaved q-LN stats ----
        psm = psA2.tile([P, 512], F32, tag="st_mean")
        pss = psA2.tile([P, 512], F32, tag="st_sq")
        for fs in range(DS):
            wqc = p2w.tile([P, DS, 128], BF16, tag="wqc")
            nc.sync.dma_start(wqc[:], wq[fs])
            ps = psA2p.tile([P, 512], F32, tag="proj")
            for ds in range(DS):
                nc.tensor.matmul(ps[:], wqc[:, ds], hT[:, ds, 0:T_OWN],
                                 start=(ds == 0), stop=(ds == DS - 1))
            for half in range(2):
                h = fs * 2 + half
                nc.vector.tensor_copy(qext[0:HD, h, :],
                                      ps[half * HD:(half + 1) * HD, :])
                qsq = p2c.tile([P, 512], F32R, tag="qsq")
                nc.scalar.activation(qsq[0:HD, :], qext[0:HD, h, :], AF.Square)
                nc.tensor.matmul(psm[:], ones128[0:HD, :], qext[0:HD, h, :],
                                 start=(h == 0), stop=(h == N_HEADS - 1))
                nc.tensor.matmul(pss[:], ones128[0:HD, :], qsq[0:HD, :],
                                 start=(h == 0), stop=(h == N_HEADS - 1))
        s_, b_ = _ln_coeffs(nc, p2s, psm, pss, 1.0 / DIM, eps_qk[:])
        for h in range(N_HEADS):
            nc.vector.tensor_tensor(qext[0:HD, h, :], qext[0:HD, h, :],
                                    s_[0:HD, :], OP.mult)
            nc.vector.tensor_tensor(qext[0:HD, h, :], qext[0:HD, h, :],
                                    b_[0:HD, :], OP.subtract)
            nc.vector.tensor_scalar_mul(qext[0:HD, h, :], qext[0:HD, h, :],
                                        qkw_sb[0:HD, h:h + 1])
        nc.sync.dma_start(qext[HD:HD + 2, :, :], qal[:])

        # ---- k projection + interleaved k-LN stats (per token chunk) ----
        kstat = []
        for tci in range(2):
            kpsm = psA2.tile([P, 512], F32, tag=f"kst_mean{tci}")
            kpss = psA2.tile([P, 512], F32, tag=f"kst_sq{tci}")
            kstat.append((kpsm, kpss))
        for fs in range(4):
            wkc = p2w.tile([P, DS, 128], BF16, tag="wkc")
            nc.sync.dma_start(wkc[:], wk[fs])
            for tci in range(2):
                tsl = slice(tci * 512, (tci + 1) * 512)
                ps = psA2p.tile([P, 512], F32, tag="proj")
                for ds in range(DS):
                    nc.tensor.matmul(ps[:], wkc[:, ds], hT[:, ds, tsl],
                                     start=(ds == 0), stop=(ds == DS - 1))
                psm, pss = kstat[tci]
                for half in range(2):
                    g = fs * 2 + half
                    nc.vector.tensor_copy(kext[0:HD, g, tsl],
                                          ps[half * HD:(half + 1) * HD, :])
                    ksq = p2c.tile([P, 512], F32R, tag="ksq")
                    nc.scalar.activation(ksq[0:HD, :], kext[0:HD, g, tsl],
                                         AF.Square)
                    nc.tensor.matmul(psm[:], ones128[0:HD, :],
                                     kext[0:HD, g, tsl],
                                     start=(g == 0), stop=(g == N_KV - 1))
                    nc.tensor.matmul(pss[:], ones128[0:HD, :], ksq[0:HD, :],
                                     start=(g == 0), stop=(g == N_KV - 1))
        for tci in range(2):
            tsl = slice(tci * 512, (tci + 1) * 512)
            psm, pss = kstat[tci]
            s_, b_ = _ln_coeffs(nc, p2s, psm, pss, 1.0 / (N_KV * HD),
                                eps_qk[:])
            for g in range(N_KV):
                nc.vector.tensor_tensor(kext[0:HD, g, tsl], kext[0:HD, g, tsl],
                                        s_[0:HD, :], OP.mult)
                nc.vector.tensor_tensor(kext[0:HD, g, tsl], kext[0:HD, g, tsl],
                                        b_[0:HD, :], OP.subtract)
        for g in range(N_KV):
            nc.sync.dma_start(kext[HD:HD + 2, g, :], kal[:])

        # ---- v projection ----
        vv0 = vext[:].rearrange("p s (g e) -> p s g e", e=HD + 1)
        nc.vector.tensor_copy(
            vv0[:, :, :, HD:HD + 1],
            ones_f[:, 0:DS * N_KV].rearrange("p (s g) -> p s g", g=N_KV)[:, :, :, None])
        wvc, free_wvc = tc.tile([P, DS, 512], BF16, name="wvc")
        nc.sync.dma_start(wvc[:], wv[:])
        vview = vext[:].rearrange("p s (g e) -> p s g e", e=HD + 1)
        for ts8 in range(DS):
            ps = psA2p.tile([P, 512], F32, tag="proj")
            for ds in range(DS):
                nc.tensor.matmul(
                    ps[:], hT[:, ds, ts8 * 128:(ts8 + 1) * 128],
                    wvc[:, ds], start=(ds == 0), stop=(ds == DS - 1))
            nc.vector.tensor_copy(
                vview[:, ts8, :, 0:HD],
                ps[:].rearrange("p (g e) -> p g e", e=HD))
        free_wvc()

    # ======================================================================
    # Phase 3: attention units (16 heads x 2 q-blocks of 256)
    # ======================================================================
    NKC = 6
    with tc.tile_pool(name="p3", bufs=3) as p3, \
         tc.tile_pool(name="psB1", bufs=2, space="PSUM") as psB1, \
         tc.tile_pool(name="psB2", bufs=1, space="PSUM") as psB2:
        for h in range(N_HEADS):
            g = h // 2
            for t2 in range(2):
                sc = psB1.tile([P, NKC * NQ], F32, tag="sc")
                for kc in range(NKC):
                    ks = t2 * 2 + kc
                    nc.tensor.matmul(
                        sc[:, kc * NQ:(kc + 1) * NQ],
                        kext[0:HD + 2, g, ks * 128:(ks + 1) * 128],
                        qext[0:HD + 2, h, t2 * NQ:(t2 + 1) * NQ],
                        start=True, stop=True)
                expS = p3.tile([P, NKC * NQ], F32R, tag="expS")
                nc.scalar.activation(expS[:], sc[:], AF.Exp, scale=0.125)
                # band mask: dist = kc*128 + r - j ; keep 0 <= dist <= 512
                for kc in (0, 1):
                    nc.gpsimd.affine_select(
                        expS[:, kc * NQ:(kc + 1) * NQ],
                        expS[:, kc * NQ:(kc + 1) * NQ],
                        pattern=[[-1, NQ]], base=kc * 128,
                        channel_multiplier=1,
                        compare_op=OP.is_ge, fill=0.0)
                for kc in (4, 5):
                    nc.gpsimd.affine_select(
                        expS[:, kc * NQ:(kc + 1) * NQ],
                        expS[:, kc * NQ:(kc + 1) * NQ],
                        pattern=[[1, NQ]], base=WINDOW - kc * 128,
                        channel_multiplier=-1,
                        compare_op=OP.is_ge, fill=0.0)
                av = psB2.tile([HD + 1, NQ], F32, tag="av")
                vv = vext[:].rearrange("p s (g e) -> p s g e", e=HD + 1)
                for kc in range(NKC):
                    ks = t2 * 2 + kc
                    nc.tensor.matmul(
                        av[:], vv[:, ks, g, :],
                        expS[:, kc * NQ:(kc + 1) * NQ],
                        start=(kc == 0), stop=(kc == NKC - 1))
                dsb = p3.tile([1, NQ], F32R, tag="dsb")
                nc.scalar.copy(dsb[:], av[HD:HD + 1, :])
                dr = psB2.tile([HD, NQ], F32, tag="dr")
                nc.tensor.matmul(dr[:], ones1[:], dsb[:],
                                 start=True, stop=True)
                rsb = p3.tile([HD, NQ], F32, tag="rsb")
                nc.vector.reciprocal(rsb[:], dr[:])
                r0 = (h % 2) * HD
                nc.vector.tensor_tensor(
                    aoT[r0:r0 + HD, h // 2, t2 * NQ:(t2 + 1) * NQ],
                    av[0:HD, :], rsb[:], OP.mult)
    free_vext()
    free_kext()
    free_qext()
    free_hT()

    # ======================================================================
    # Phase 4: wo projection + residual -> x2T ; ffn LN -> h2T
    # ======================================================================
    h2T = tile_single([P, DS, T_OWN], BF16, name="h2T")
    with tc.tile_pool(name="p4", bufs=2) as p4, \
         tc.tile_pool(name="p4s", bufs=1) as p4s, \
         tc.tile_pool(name="psC", bufs=2, space="PSUM") as psC:
        for ds2 in range(DS):
            ps = psC.tile([P, 512], F32, tag="proj")
            for hs8 in range(KS):
                nc.tensor.matmul(
                    ps[:], woc[:, hs8, ds2 * 128:(ds2 + 1) * 128],
                    aoT[:, hs8], start=(hs8 == 0), stop=(hs8 == KS - 1))
            t_ = p4.tile([P, 512], F32, tag="t")
            nc.vector.tensor_scalar_mul(t_[:], ps[:], asc_sb[:, ds2:ds2 + 1])
            nc.vector.tensor_tensor(x2T[:, ds2], t_[:], xTo[:, ds2], OP.add)

        # ffn LN (ffn_norm_w folded into w1/w3)
        psm = psC.tile([P, 512], F32, tag="st_mean")
        pss = psC.tile([P, 512], F32, tag="st_sq")
        for ds in range(DS):
            xr = p4.tile([P, 512], F32R, tag="xr")
            nc.vector.tensor_copy(xr[:], x2T[:, ds])
            nc.tensor.matmul(psm[:], ones128[:], xr[:],
                             start=(ds == 0), stop=(ds == DS - 1))
        for ds in range(DS):
            xq = p4.tile([P, 512], F32R, tag="xq")
            nc.scalar.activation(xq[:], x2T[:, ds], AF.Square)
            nc.tensor.matmul(pss[:], ones128[:], xq[:],
                             start=(ds == 0), stop=(ds == DS - 1))
        s_, b_ = _ln_coeffs(nc, p4s, psm, pss, 1.0 / DIM, eps_n[:])
        for ds in range(DS):
            t_ = p4.tile([P, 512], F32, tag="t")
            nc.vector.tensor_tensor(t_[:], x2T[:, ds], s_[:], OP.mult)
            nc.vector.tensor_tensor(h2T[:, ds], t_[:], b_[:], OP.subtract)

    # ======================================================================
    # Phase 5: SwiGLU FFN + residual -> out
    # ======================================================================
    gT, free_gT = tc.tile([P, HS, T_OWN], BF16, name="gT")
    with tc.tile_pool(name="p5", bufs=3) as p5, \
         tc.tile_pool(name="p5w2", bufs=2) as p5w2, \
         tc.tile_pool(name="psD", bufs=1, space="PSUM") as psD, \
         tc.tile_pool(name="psDy", bufs=2, space="PSUM") as psDy:
        for hs2 in range(HS // 2):
            w1c = p5.tile([P, DS, 256], BF16, tag="w1c")
            w3c = p5.tile([P, DS, 256], BF16, tag="w3c")
            for half in range(2):
                fsl = slice(half * 128, (half + 1) * 128)
                nc.sync.dma_start(w1c[:, :, fsl], w1[hs2 * 2 + half])
                nc.sync.dma_start(w3c[:, :, fsl], w3[hs2 * 2 + half])
            psu = psD.tile([P, 1024], F32, tag="u")
            psw = psD.tile([P, 1024], F32, tag="w")
            for half in range(2):
                for ds in range(DS):
                    nc.tensor.matmul(psu[:, half * 512:(half + 1) * 512],
                                     w1c[:, ds, half * 128:(half + 1) * 128],
                                     h2T[:, ds],
                                     start=(ds == 0), stop=(ds == DS - 1))
                for ds in range(DS):
                    nc.tensor.matmul(psw[:, half * 512:(half + 1) * 512],
                                     w3c[:, ds, half * 128:(half + 1) * 128],
                                     h2T[:, ds],
                                     start=(ds == 0), stop=(ds == DS - 1))
            sil = p5.tile([P, 1024], F32, tag="sil")
            nc.scalar.activation(sil[:], psu[:], AF.Silu)
            nc.vector.tensor_tensor(
                gT[:, hs2 * 2:hs2 * 2 + 2].rearrange("p a b -> p (a b)"),
                sil[:], psw[:], OP.mult)

        for ds2 in range(DS):
            w2c = p5w2.tile([P, HS, 128], BF16, tag="w2c")
            nc.sync.dma_start(w2c[:], w2[ds2])
            psy = psDy.tile([P, 512], F32, tag="y")
            for hs in range(HS):
                nc.tensor.matmul(psy[:], w2c[:, hs], gT[:, hs],
                                 start=(hs == 0), stop=(hs == HS - 1))
            t_ = p5.tile([P, 512], F32, tag="t")
            nc.vector.tensor_scalar_mul(t_[:], psy[:], fsc_sb[:, ds2:ds2 + 1])
            yv = p5.tile([P, 512], F32, tag="yv")
            nc.vector.tensor_tensor(yv[:], t_[:], x2T[:, ds2], OP.add)
            nc.sync.dma_start(out[:, ds2, :], yv[:])
    free_gT()
    for f in reversed(frees):
        f()


# ---------------------------------------------------------------------------
# host side
# ---------------------------------------------------------------------------

def _tile_kxf(wT, f_chunk):
    """[K, F] (K=contraction, multiple of 128) -> [F//f_chunk, 128, K//128,
    f_chunk] chunks whose DMA into SBUF [p, ksub, f_chunk] is contiguous."""
    K, F = wT.shape
    return np.ascontiguousarray(
        wT.reshape(K // P, P, F // f_chunk, f_chunk).transpose(2, 1, 0, 3))


def _prep_inputs(x, wq, wk, wv, wo, q_norm_w, k_norm_w, attn_norm_w,
                 ffn_norm_w, w1, w2, w3, attn_scale, ffn_scale):
    bf = ml_dtypes.bfloat16
    x = np.asarray(x, np.float32)
    wq = np.asarray(wq, np.float32)
    wk = np.asarray(wk, np.float32)
    wv = np.asarray(wv, np.float32)
    wo = np.asarray(wo, np.float32)
    w1 = np.asarray(w1, np.float32)
    w2 = np.asarray(w2, np.float32)
    w3 = np.asarray(w3, np.float32)
    q_norm_w = np.asarray(q_norm_w, np.float32)
    k_norm_w = np.asarray(k_norm_w, np.float32)
    attn_norm_w = np.asarray(attn_norm_w, np.float32)
    ffn_norm_w = np.asarray(ffn_norm_w, np.float32)
    attn_scale = np.asarray(attn_scale, np.float32)
    ffn_scale = np.asarray(ffn_scale, np.float32)

    # fold attn_norm into wq/wk/wv, ffn_norm into w1/w3 (column scales)
    wq_e = wq * attn_norm_w[None, :]
    wk_e = wk * attn_norm_w[None, :]
    wv_e = wv * attn_norm_w[None, :]
    w1_e = w1 * ffn_norm_w[None, :]
    w3_e = w3 * ffn_norm_w[None, :]

    wq_t = _tile_kxf(wq_e.T, 128).astype(bf)           # [8,128,8,128]
    wk_t = _tile_kxf(wk_e.T, 128).astype(bf)           # [4,128,8,128]
    wv_t = np.ascontiguousarray(
        wv_e.T.reshape(DS, P, 512).transpose(1, 0, 2)).astype(bf)
    wo_t = np.ascontiguousarray(
        wo.T.reshape(KS, P, DIM).transpose(1, 0, 2)).astype(bf)
    w1_t = _tile_kxf(w1_e.T, 128).astype(bf)           # [32,128,8,128]
    w3_t = _tile_kxf(w3_e.T, 128).astype(bf)
    w2_t = _tile_kxf(w2.T, 128).astype(bf)             # [8,128,32,128]

    # qnw*knw folded, head-local layout [p(<64), h]
    qkw = np.zeros((P, N_HEADS), np.float32)
    for h in range(N_HEADS):
        qf = h * HD + np.arange(HD)
        kf = (h // 2) * HD + np.arange(HD)
        qkw[0:HD, h] = q_norm_w[qf] * k_norm_w[kf]

    def vec_tile(v):
        return np.ascontiguousarray(v.reshape(DS, P).T)

    asc = vec_tile(attn_scale)
    fsc = vec_tile(ffn_scale)

    per_core = []
    for c in range(8):
        b, blk = c // 4, c % 4
        q0 = blk * T_OWN
        hi = min(q0 + T_HALO, S)
        xblk = np.zeros((T_HALO, DIM), np.float32)
        xblk[0:hi - q0] = x[b, q0:hi]
        xT = np.ascontiguousarray(
            xblk.T.reshape(DS, P, T_HALO).transpose(1, 0, 2))
        kidx = np.arange(T_HALO, dtype=np.float32)
        if hi - q0 < T_HALO:
            kidx[hi - q0:] += 1e9
        kal = np.stack([kidx, np.ones(T_HALO, np.float32)])
        qal = np.empty((2, N_HEADS, T_OWN), np.float32)
        for h in range(N_HEADS):
            qal[0, h, :] = -8.0 * SLOPES[h]
            qal[1, h, :] = 8.0 * SLOPES[h] * np.arange(T_OWN)
        per_core.append({
            "xT": xT, "wq": wq_t, "wk": wk_t, "wv": wv_t, "wo": wo_t,
            "w1": w1_t, "w3": w3_t, "w2": w2_t, "qkw": qkw,
            "asc": asc, "fsc": fsc, "kal": kal, "qal": qal,
        })
    return per_core


_NC_CACHE = None
LAST_RESULT = None  # BassKernelResults of the most recent run (for profiling)
TRACE = False


def kernel(**inputs):
    global _NC_CACHE, LAST_RESULT
    per_core = _prep_inputs(**inputs)
    if _NC_CACHE is None:
        _NC_CACHE = _build_nc()
    res = run_bass_kernel_spmd(_NC_CACHE, per_core, core_ids=list(range(8)),
                               trace=TRACE)
    LAST_RESULT = res
    full = np.empty((B, S, DIM), np.float32)
    for c in range(8):
        b, blk = c // 4, c % 4
        y = res.results[c]["out"]                      # [p, ds, tok]
        full[b, blk * T_OWN:(blk + 1) * T_OWN] = (
            y.transpose(2, 1, 0).reshape(T_OWN, DIM))
    return full



# revision 11
# speedup vs baseline: 1.1298x; 1.1298x over previous
"""Trainium2 Bass kernel for nn_CodecTransformerLayer (sparse window attention
+ GQA + ALiBi + SwiGLU FFN), 8-core data-parallel with forward-halo recompute.

Sharding: batch(2) x seq-block(4) = 8 shards, one per core. Each core computes
its own 512 tokens end-to-end; attention needs K/V for the next 512 tokens
(window is forward-looking: dist = j - i in [0, 512]), which the core
recomputes from a 512-token halo of x instead of communicating.

Layout: feature-on-partition ("transposed") activations everywhere. All
weights and x are pre-transposed/pre-tiled on the host into the exact SBUF
layouts, so every DMA is contiguous. LayerNorm partition-dim reductions use
all-ones matmuls (gives the mean replicated across partitions for free).
ALiBi + band mask enter the score matmul as two extra contraction rows
(rank-2 decomposition of the in-band mask); out-of-band positions are zeroed
on the exp output with affine_select; invalid halo keys (last block of each
batch) get a +1e9 key-index so their logit is ~-1e9.

Matmul dtypes: bf16 for projections/FFN (error is scaled by 1e-5 residual
scales), float32r (reduced-mantissa fp32, full PE rate at N>=256) for
attention scores/AV and LN statistics. Residual path stays exact fp32.
"""

import math

import numpy as np
import ml_dtypes

import concourse.bass as bass
import concourse.mybir as mybir
import concourse.tile as tile
from concourse import bacc
from concourse.bass_utils import run_bass_kernel_spmd

P = 128
DIM = 1024
N_HEADS = 16
N_KV = 8
HD = 64
HIDDEN = 4096
WINDOW = 512
NORM_EPS = 1e-5
QK_EPS = 1e-6
B = 2
S = 2048
T_OWN = 512          # tokens owned per core
T_HALO = 1024        # own + forward halo
DS = DIM // P        # 8 d-subtiles
KS = DIM // P        # 8 hd-subtiles for wo contraction
HS = HIDDEN // P     # 32 hidden subtiles

F32 = mybir.dt.float32
F32R = mybir.dt.float32r
BF16 = mybir.dt.bfloat16
F8 = mybir.dt.float8e4
DR = mybir.MatmulPerfMode.DoubleRow
AF = mybir.ActivationFunctionType
OP = mybir.AluOpType

# fp8 weight scales (folded out via silu scale / host-side fsc fold)
S1 = 16.0
S3 = 16.0
S2 = 64.0


def _alibi_slopes(n):
    start = 2.0 ** (-(2.0 ** (-(math.log2(n) - 3))))
    return [start * start ** i for i in range(n)]


SLOPES = _alibi_slopes(N_HEADS)


# ---------------------------------------------------------------------------
# device kernel
# ---------------------------------------------------------------------------

def _build_nc():
    nc = bacc.Bacc("TRN2")

    ins = {}
    ins["xT"] = nc.dram_tensor("xT", [P, DS, T_HALO], F32, kind="ExternalInput")
    ins["wq"] = nc.dram_tensor("wq", [8, P, DS, 128], BF16, kind="ExternalInput")
    ins["wk"] = nc.dram_tensor("wk", [4, P, DS, 128], BF16, kind="ExternalInput")
    ins["wv"] = nc.dram_tensor("wv", [P, DS, 512], BF16, kind="ExternalInput")
    ins["wo"] = nc.dram_tensor("wo", [P, KS, DIM], BF16, kind="ExternalInput")
    ins["w1"] = nc.dram_tensor("w1", [HS, P, DS, 128], F8, kind="ExternalInput")
    ins["w3"] = nc.dram_tensor("w3", [HS, P, DS, 128], F8, kind="ExternalInput")
    ins["w2"] = nc.dram_tensor("w2", [DS, P, HS, 128], F8, kind="ExternalInput")
    # qnw*knw folded, head-local layout [64(pad128), head]
    ins["qkw"] = nc.dram_tensor("qkw", [P, N_HEADS], F32, kind="ExternalInput")
    ins["asc"] = nc.dram_tensor("asc", [P, DS], F32, kind="ExternalInput")
    ins["fsc"] = nc.dram_tensor("fsc", [P, DS], F32, kind="ExternalInput")
    ins["kal"] = nc.dram_tensor("kal", [2, T_HALO], F32R, kind="ExternalInput")
    ins["qal"] = nc.dram_tensor("qal", [2, N_HEADS, T_OWN], F32R, kind="ExternalInput")

    out = nc.dram_tensor("out", [P, DS, T_OWN], F32, kind="ExternalOutput")

    with tile.TileContext(nc) as tc:
        _emit(nc, tc, ins, out)
    nc.finalize()
    return nc


def _ln_coeffs(nc, pool, psm, pss, inv_n, eps_ap):
    """From sum/sumsq psums (replicated across partitions), produce
    a = rstd and b = mean * rstd, both [128, 512] f32 replicated."""
    m_ = pool.tile([P, 512], F32, tag="ln_m")
    nc.vector.tensor_scalar_mul(m_[:], psm[:], inv_n)
    v_ = pool.tile([P, 512], F32, tag="ln_v")
    nc.vector.tensor_scalar_mul(v_[:], pss[:], inv_n)
    mm_ = pool.tile([P, 512], F32, tag="ln_mm")
    nc.vector.tensor_tensor(mm_[:], m_[:], m_[:], OP.mult)
    nc.vector.tensor_tensor(v_[:], v_[:], mm_[:], OP.subtract)
    s_ = pool.tile([P, 512], F32, tag="ln_s")
    nc.scalar.activation(s_[:], v_[:], AF.Sqrt, bias=eps_ap)
    nc.vector.reciprocal(s_[:], s_[:])
    b_ = pool.tile([P, 512], F32, tag="ln_b")
    nc.vector.tensor_tensor(b_[:], m_[:], s_[:], OP.mult)
    return s_, b_


def _emit(nc, tc, ins, out):
    frees = []  # keep single-tile pool handles alive; release LIFO at end

    def tile_single(shape, dtype, name):
        t, f = tc.tile(shape, dtype, name=name)
        frees.append(f)
        return t

    xT, wq, wk, wv, wo = ins["xT"], ins["wq"], ins["wk"], ins["wv"], ins["wo"]
    w1, w3, w2 = ins["w1"], ins["w3"], ins["w2"]
    qkw, asc, fsc = ins["qkw"], ins["asc"], ins["fsc"]
    kal, qal = ins["kal"], ins["qal"]

    # --- constants (kept for the whole kernel) -----------------------------
    ones_f = tile_single([P, P], F32, name="ones_f")
    nc.vector.memset(ones_f[:], 1.0)
    ones128 = tile_single([P, P], F32R, name="ones128")
    nc.vector.tensor_copy(ones128[:], ones_f[:])
    ones1 = tile_single([1, HD], F32R, name="ones1")
    nc.vector.tensor_copy(ones1[:], ones_f[0:1, 0:HD])
    qkw_sb = tile_single([P, N_HEADS], F32, name="qkw_sb")
    nc.sync.dma_start(qkw_sb[:], qkw[:])
    asc_sb = tile_single([P, DS], F32, name="asc_sb")
    nc.sync.dma_start(asc_sb[:], asc[:])
    fsc_sb = tile_single([P, DS], F32, name="fsc_sb")
    nc.sync.dma_start(fsc_sb[:], fsc[:])
    eps_n = tile_single([P, 1], F32, name="eps_n")
    nc.vector.memset(eps_n[:], NORM_EPS)
    eps_qk = tile_single([P, 1], F32, name="eps_qk")
    nc.vector.memset(eps_qk[:], QK_EPS)

    xTo = tile_single([P, DS, T_OWN], F32, name="xTo")
    nc.sync.dma_start(xTo[:], xT[:, :, 0:T_OWN])
    aoT = tile_single([P, KS, T_OWN], BF16, name="aoT")
    x2T = tile_single([P, DS, T_OWN], F32, name="x2T")

    woc = tile_single([P, KS, DIM], BF16, name="woc")
    nc.sync.dma_start(woc[:], wo[:])

    NQ = 256

    hT, free_hT = tc.tile([P, DS, T_HALO], BF16, name="hT")

    # ======================================================================
    # Phase 1: attn LN over halo tokens -> hT (bf16)
    # (attn_norm_w is folded into wq/wk/wv on the host)
    # ======================================================================
    xTh, free_xTh = tc.tile([P, DS, T_OWN], F32, name="xTh")
    nc.sync.dma_start(xTh[:], xT[:, :, T_OWN:T_HALO])
    with tc.tile_pool(name="p1c", bufs=3) as p1c, \
         tc.tile_pool(name="p1s", bufs=1) as p1s, \
         tc.tile_pool(name="psA1", bufs=2, space="PSUM") as psA1:
        for tci, xsrc in ((0, xTo), (1, xTh)):
            psm = psA1.tile([P, 512], F32, tag="st_mean")
            pss = psA1.tile([P, 512], F32, tag="st_sq")
            for ds in range(DS):
                xr = p1c.tile([P, 512], F32R, tag="xr")
                nc.vector.tensor_copy(xr[:], xsrc[:, ds])
                nc.tensor.matmul(psm[:], ones128[:], xr[:],
                                 start=(ds == 0), stop=(ds == DS - 1))
            for ds in range(DS):
                xq = p1c.tile([P, 512], F32R, tag="xq")
                nc.scalar.activation(xq[:], xsrc[:, ds], AF.Square)
                nc.tensor.matmul(pss[:], ones128[:], xq[:],
                                 start=(ds == 0), stop=(ds == DS - 1))
            s_, b_ = _ln_coeffs(nc, p1s, psm, pss, 1.0 / DIM, eps_n[:])
            for ds in range(DS):
                t_ = p1c.tile([P, 512], F32, tag="t")
                nc.vector.tensor_tensor(t_[:], xsrc[:, ds], s_[:], OP.mult)
                nc.vector.tensor_tensor(
                    hT[:, ds, tci * 512:(tci + 1) * 512], t_[:], b_[:],
                    OP.subtract)
    free_xTh()

    # ======================================================================
    # Phase 2: q/k/v projections + q/k LN (in-place) -> qext, kext, vext
    # qext[h]: rows 0..63 = q_ln (head h), row 64 = -8*slope, row 65 =
    # 8*slope*qidx. kext[g]: rows 0..63 = k_ln, row 64 = kidx, row 65 = 1.
    # vext: [tok_p, tok_sub, kv*(HD+1)] with a ones column per kv head.
    # ======================================================================
    qext, free_qext = tc.tile([P, N_HEADS, T_OWN], F32R, name="qext")
    kext, free_kext = tc.tile([P, N_KV, T_HALO], F32R, name="kext")
    vext, free_vext = tc.tile([P, DS, N_KV * (HD + 1)], F32R, name="vext")

    with tc.tile_pool(name="p2w", bufs=3) as p2w, \
         tc.tile_pool(name="p2c", bufs=2) as p2c, \
         tc.tile_pool(name="p2s", bufs=1) as p2s, \
         tc.tile_pool(name="psA2", bufs=1, space="PSUM") as psA2, \
         tc.tile_pool(name="psA2p", bufs=2, space="PSUM") as psA2p:

        # ---- q projection + interleaved q-LN stats ----
        psm = psA2.tile([P, 512], F32, tag="st_mean")
        pss = psA2.tile([P, 512], F32, tag="st_sq")
        for fs in range(DS):
            wqc = p2w.tile([P, DS, 128], BF16, tag="wqc")
            nc.sync.dma_start(wqc[:], wq[fs])
            ps = psA2p.tile([P, 512], F32, tag="proj")
            for ds in range(DS):
                nc.tensor.matmul(ps[:], wqc[:, ds], hT[:, ds, 0:T_OWN],
                                 start=(ds == 0), stop=(ds == DS - 1))
            for half in range(2):
                h = fs * 2 + half
                nc.vector.tensor_copy(qext[0:HD, h, :],
                                      ps[half * HD:(half + 1) * HD, :])
                qsq = p2c.tile([P, 512], F32R, tag="qsq")
                nc.scalar.activation(qsq[0:HD, :], qext[0:HD, h, :], AF.Square)
                nc.tensor.matmul(psm[:], ones128[0:HD, :], qext[0:HD, h, :],
                                 start=(h == 0), stop=(h == N_HEADS - 1))
                nc.tensor.matmul(pss[:], ones128[0:HD, :], qsq[0:HD, :],
                                 start=(h == 0), stop=(h == N_HEADS - 1))
        s_, b_ = _ln_coeffs(nc, p2s, psm, pss, 1.0 / DIM, eps_qk[:])
        for h in range(N_HEADS):
            nc.vector.tensor_tensor(qext[0:HD, h, :], qext[0:HD, h, :],
                                    s_[0:HD, :], OP.mult)
            nc.vector.tensor_tensor(qext[0:HD, h, :], qext[0:HD, h, :],
                                    b_[0:HD, :], OP.subtract)
            nc.vector.tensor_scalar_mul(qext[0:HD, h, :], qext[0:HD, h, :],
                                        qkw_sb[0:HD, h:h + 1])
        nc.sync.dma_start(qext[HD:HD + 2, :, :], qal[:])

        # ---- k projection + interleaved k-LN stats (per token chunk) ----
        kstat = []
        for tci in range(2):
            kpsm = psA2.tile([P, 512], F32, tag=f"kst_mean{tci}")
            kpss = psA2.tile([P, 512], F32, tag=f"kst_sq{tci}")
            kstat.append((kpsm, kpss))
        for fs in range(4):
            wkc = p2w.tile([P, DS, 128], BF16, tag="wkc")
            nc.sync.dma_start(wkc[:], wk[fs])
            for tci in range(2):
                tsl = slice(tci * 512, (tci + 1) * 512)
                ps = psA2p.tile([P, 512], F32, tag="proj")
                for ds in range(DS):
                    nc.tensor.matmul(ps[:], wkc[:, ds], hT[:, ds, tsl],
                                     start=(ds == 0), stop=(ds == DS - 1))
                psm, pss = kstat[tci]
                for half in range(2):
                    g = fs * 2 + half
                    nc.vector.tensor_copy(kext[0:HD, g, tsl],
                                          ps[half * HD:(half + 1) * HD, :])
                    ksq = p2c.tile([P, 512], F32R, tag="ksq")
                    nc.scalar.activation(ksq[0:HD, :], kext[0:HD, g, tsl],
                                         AF.Square)
                    nc.tensor.matmul(psm[:], ones128[0:HD, :],
                                     kext[0:HD, g, tsl],
                                     start=(g == 0), stop=(g == N_KV - 1))
                    nc.tensor.matmul(pss[:], ones128[0:HD, :], ksq[0:HD, :],
                                     start=(g == 0), stop=(g == N_KV - 1))
        for tci in range(2):
            tsl = slice(tci * 512, (tci + 1) * 512)
            psm, pss = kstat[tci]
            s_, b_ = _ln_coeffs(nc, p2s, psm, pss, 1.0 / (N_KV * HD),
                                eps_qk[:])
            for g in range(N_KV):
                nc.vector.tensor_tensor(kext[0:HD, g, tsl], kext[0:HD, g, tsl],
                                        s_[0:HD, :], OP.mult)
                nc.vector.tensor_tensor(kext[0:HD, g, tsl], kext[0:HD, g, tsl],
                                        b_[0:HD, :], OP.subtract)
        for g in range(N_KV):
            nc.sync.dma_start(kext[HD:HD + 2, g, :], kal[:])

        # ---- v projection ----
        vv0 = vext[:].rearrange("p s (g e) -> p s g e", e=HD + 1)
        nc.vector.tensor_copy(
            vv0[:, :, :, HD:HD + 1],
            ones_f[:, 0:DS * N_KV].rearrange("p (s g) -> p s g", g=N_KV)[:, :, :, None])
        wvc, free_wvc = tc.tile([P, DS, 512], BF16, name="wvc")
        nc.sync.dma_start(wvc[:], wv[:])
        vview = vext[:].rearrange("p s (g e) -> p s g e", e=HD + 1)
        for ts8 in range(DS):
            ps = psA2p.tile([P, 512], F32, tag="proj")
            for ds in range(DS):
                nc.tensor.matmul(
                    ps[:], hT[:, ds, ts8 * 128:(ts8 + 1) * 128],
                    wvc[:, ds], start=(ds == 0), stop=(ds == DS - 1))
            nc.vector.tensor_copy(
                vview[:, ts8, :, 0:HD],
                ps[:].rearrange("p (g e) -> p g e", e=HD))
        free_wvc()

    # ======================================================================
    # Phase 3: attention units (16 heads x 2 q-blocks of 256)
    # ======================================================================
    NKC = 6
    with tc.tile_pool(name="p3", bufs=3) as p3, \
         tc.tile_pool(name="psB1", bufs=2, space="PSUM") as psB1, \
         tc.tile_pool(name="psB2", bufs=1, space="PSUM") as psB2:
        for h in range(N_HEADS):
            g = h // 2
            for t2 in range(2):
                sc = psB1.tile([P, NKC * NQ], F32, tag="sc")
                for kc in range(NKC):
                    ks = t2 * 2 + kc
                    nc.tensor.matmul(
                        sc[:, kc * NQ:(kc + 1) * NQ],
                        kext[0:HD + 2, g, ks * 128:(ks + 1) * 128],
                        qext[0:HD + 2, h, t2 * NQ:(t2 + 1) * NQ],
                        start=True, stop=True)
                expS = p3.tile([P, NKC * NQ], F32R, tag="expS")
                nc.scalar.activation(expS[:], sc[:], AF.Exp, scale=0.125)
                # band mask: dist = kc*128 + r - j ; keep 0 <= dist <= 512
                for kc in (0, 1):
                    nc.gpsimd.affine_select(
                        expS[:, kc * NQ:(kc + 1) * NQ],
                        expS[:, kc * NQ:(kc + 1) * NQ],
                        pattern=[[-1, NQ]], base=kc * 128,
                        channel_multiplier=1,
                        compare_op=OP.is_ge, fill=0.0)
                for kc in (4, 5):
                    nc.gpsimd.affine_select(
                        expS[:, kc * NQ:(kc + 1) * NQ],
                        expS[:, kc * NQ:(kc + 1) * NQ],
                        pattern=[[1, NQ]], base=WINDOW - kc * 128,
                        channel_multiplier=-1,
                        compare_op=OP.is_ge, fill=0.0)
                av = psB2.tile([HD + 1, NQ], F32, tag="av")
                vv = vext[:].rearrange("p s (g e) -> p s g e", e=HD + 1)
                for kc in range(NKC):
                    ks = t2 * 2 + kc
                    nc.tensor.matmul(
                        av[:], vv[:, ks, g, :],
                        expS[:, kc * NQ:(kc + 1) * NQ],
                        start=(kc == 0), stop=(kc == NKC - 1))
                dsb = p3.tile([1, NQ], F32R, tag="dsb")
                nc.scalar.copy(dsb[:], av[HD:HD + 1, :])
                dr = psB2.tile([HD, NQ], F32, tag="dr")
                nc.tensor.matmul(dr[:], ones1[:], dsb[:],
                                 start=True, stop=True)
                rsb = p3.tile([HD, NQ], F32, tag="rsb")
                nc.vector.reciprocal(rsb[:], dr[:])
                r0 = (h % 2) * HD
                nc.vector.tensor_tensor(
                    aoT[r0:r0 + HD, h // 2, t2 * NQ:(t2 + 1) * NQ],
                    av[0:HD, :], rsb[:], OP.mult)
    free_vext()
    free_kext()
    free_qext()
    free_hT()

    # ======================================================================
    # Phase 4: wo projection + residual -> x2T ; ffn LN -> h2T
    # ======================================================================
    h2T = tile_single([P, DS, T_OWN], F8, name="h2T")
    with tc.tile_pool(name="p4", bufs=2) as p4, \
         tc.tile_pool(name="p4s", bufs=1) as p4s, \
         tc.tile_pool(name="psC", bufs=2, space="PSUM") as psC:
        for ds2 in range(DS):
            ps = psC.tile([P, 512], F32, tag="proj")
            for hs8 in range(KS):
                nc.tensor.matmul(
                    ps[:], woc[:, hs8, ds2 * 128:(ds2 + 1) * 128],
                    aoT[:, hs8], start=(hs8 == 0), stop=(hs8 == KS - 1))
            t_ = p4.tile([P, 512], F32, tag="t")
            nc.vector.tensor_scalar_mul(t_[:], ps[:], asc_sb[:, ds2:ds2 + 1])
            nc.vector.tensor_tensor(x2T[:, ds2], t_[:], xTo[:, ds2], OP.add)

        # ffn LN (ffn_norm_w folded into w1/w3)
        psm = psC.tile([P, 512], F32, tag="st_mean")
        pss = psC.tile([P, 512], F32, tag="st_sq")
        for ds in range(DS):
            xr = p4.tile([P, 512], F32R, tag="xr")
            nc.vector.tensor_copy(xr[:], x2T[:, ds])
            nc.tensor.matmul(psm[:], ones128[:], xr[:],
                             start=(ds == 0), stop=(ds == DS - 1))
        for ds in range(DS):
            xq = p4.tile([P, 512], F32R, tag="xq")
            nc.scalar.activation(xq[:], x2T[:, ds], AF.Square)
            nc.tensor.matmul(pss[:], ones128[:], xq[:],
                             start=(ds == 0), stop=(ds == DS - 1))
        s_, b_ = _ln_coeffs(nc, p4s, psm, pss, 1.0 / DIM, eps_n[:])
        for ds in range(DS):
            t_ = p4.tile([P, 512], F32, tag="t")
            nc.vector.tensor_tensor(t_[:], x2T[:, ds], s_[:], OP.mult)
            nc.vector.tensor_tensor(h2T[:, ds], t_[:], b_[:], OP.subtract)

    # ======================================================================
    # Phase 5: SwiGLU FFN + residual -> out
    # ======================================================================
    gT, free_gT = tc.tile([P, HS, T_OWN], F8, name="gT")
    with tc.tile_pool(name="p5", bufs=3) as p5, \
         tc.tile_pool(name="p5w2", bufs=2) as p5w2, \
         tc.tile_pool(name="psD", bufs=1, space="PSUM") as psD, \
         tc.tile_pool(name="psDy", bufs=2, space="PSUM") as psDy:
        for hs2 in range(HS // 2):
            w1c = p5.tile([P, DS, 256], F8, tag="w1c")
            w3c = p5.tile([P, DS, 256], F8, tag="w3c")
            for half in range(2):
                fsl = slice(half * 128, (half + 1) * 128)
                nc.sync.dma_start(w1c[:, :, fsl], w1[hs2 * 2 + half])
                nc.sync.dma_start(w3c[:, :, fsl], w3[hs2 * 2 + half])
            psu = psD.tile([P, 1024], F32, tag="u")
            psw = psD.tile([P, 1024], F32, tag="w")
            for half in range(2):
                fsl = slice(half * 128, (half + 1) * 128)
                for sup in range(DS // 2):
                    nc.tensor.matmul(psu[:, half * 512:(half + 1) * 512],
                                     w1c[:, 2 * sup:2 * sup + 2, fsl],
                                     h2T[:, 2 * sup:2 * sup + 2, :],
                                     start=(sup == 0), stop=(sup == DS // 2 - 1),
                                     perf_mode=DR)
                for sup in range(DS // 2):
                    nc.tensor.matmul(psw[:, half * 512:(half + 1) * 512],
                                     w3c[:, 2 * sup:2 * sup + 2, fsl],
                                     h2T[:, 2 * sup:2 * sup + 2, :],
                                     start=(sup == 0), stop=(sup == DS // 2 - 1),
                                     perf_mode=DR)
            sil = p5.tile([P, 1024], F32, tag="sil")
            nc.scalar.activation(sil[:], psu[:], AF.Silu, scale=1.0 / S1)
            nc.vector.tensor_tensor(
                gT[:, hs2 * 2:hs2 * 2 + 2].rearrange("p a b -> p (a b)"),
                sil[:], psw[:], OP.mult)

        for ds2 in range(DS):
            w2c = p5w2.tile([P, HS, 128], F8, tag="w2c")
            nc.sync.dma_start(w2c[:], w2[ds2])
            psy = psDy.tile([P, 512], F32, tag="y")
            for sup in range(HS // 2):
                nc.tensor.matmul(psy[:], w2c[:, 2 * sup:2 * sup + 2, :],
                                 gT[:, 2 * sup:2 * sup + 2, :],
                                 start=(sup == 0), stop=(sup == HS // 2 - 1),
                                 perf_mode=DR)
            t_ = p5.tile([P, 512], F32, tag="t")
            nc.vector.tensor_scalar_mul(t_[:], psy[:], fsc_sb[:, ds2:ds2 + 1])
            yv = p5.tile([P, 512], F32, tag="yv")
            nc.vector.tensor_tensor(yv[:], t_[:], x2T[:, ds2], OP.add)
            nc.sync.dma_start(out[:, ds2, :], yv[:])
    free_gT()
    for f in reversed(frees):
        f()


# ---------------------------------------------------------------------------
# host side
# ---------------------------------------------------------------------------

def _tile_kxf(wT, f_chunk):
    """[K, F] (K=contraction, multiple of 128) -> [F//f_chunk, 128, K//128,
    f_chunk] chunks whose DMA into SBUF [p, ksub, f_chunk] is contiguous."""
    K, F = wT.shape
    return np.ascontiguousarray(
        wT.reshape(K // P, P, F // f_chunk, f_chunk).transpose(2, 1, 0, 3))


def _prep_inputs(x, wq, wk, wv, wo, q_norm_w, k_norm_w, attn_norm_w,
                 ffn_norm_w, w1, w2, w3, attn_scale, ffn_scale):
    bf = ml_dtypes.bfloat16
    x = np.asarray(x, np.float32)
    wq = np.asarray(wq, np.float32)
    wk = np.asarray(wk, np.float32)
    wv = np.asarray(wv, np.float32)
    wo = np.asarray(wo, np.float32)
    w1 = np.asarray(w1, np.float32)
    w2 = np.asarray(w2, np.float32)
    w3 = np.asarray(w3, np.float32)
    q_norm_w = np.asarray(q_norm_w, np.float32)
    k_norm_w = np.asarray(k_norm_w, np.float32)
    attn_norm_w = np.asarray(attn_norm_w, np.float32)
    ffn_norm_w = np.asarray(ffn_norm_w, np.float32)
    attn_scale = np.asarray(attn_scale, np.float32)
    ffn_scale = np.asarray(ffn_scale, np.float32)

    # fold attn_norm into wq/wk/wv, ffn_norm into w1/w3 (column scales)
    wq_e = wq * attn_norm_w[None, :]
    wk_e = wk * attn_norm_w[None, :]
    wv_e = wv * attn_norm_w[None, :]
    w1_e = w1 * ffn_norm_w[None, :]
    w3_e = w3 * ffn_norm_w[None, :]

    wq_t = _tile_kxf(wq_e.T, 128).astype(bf)           # [8,128,8,128]
    wk_t = _tile_kxf(wk_e.T, 128).astype(bf)           # [4,128,8,128]
    wv_t = np.ascontiguousarray(
        wv_e.T.reshape(DS, P, 512).transpose(1, 0, 2)).astype(bf)
    wo_t = np.ascontiguousarray(
        wo.T.reshape(KS, P, DIM).transpose(1, 0, 2)).astype(bf)
    f8 = ml_dtypes.float8_e4m3
    w1_t = _tile_kxf(w1_e.T * S1, 128).astype(f8)      # [32,128,8,128]
    w3_t = _tile_kxf(w3_e.T * S3, 128).astype(f8)
    w2_t = _tile_kxf(w2.T * S2, 128).astype(f8)        # [8,128,32,128]

    # qnw*knw folded, head-local layout [p(<64), h]
    qkw = np.zeros((P, N_HEADS), np.float32)
    for h in range(N_HEADS):
        qf = h * HD + np.arange(HD)
        kf = (h // 2) * HD + np.arange(HD)
        qkw[0:HD, h] = q_norm_w[qf] * k_norm_w[kf]

    def vec_tile(v):
        return np.ascontiguousarray(v.reshape(DS, P).T)

    asc = vec_tile(attn_scale)
    fsc = vec_tile(ffn_scale) / (S2 * S3)

    per_core = []
    for c in range(8):
        b, blk = c // 4, c % 4
        q0 = blk * T_OWN
        hi = min(q0 + T_HALO, S)
        xblk = np.zeros((T_HALO, DIM), np.float32)
        xblk[0:hi - q0] = x[b, q0:hi]
        xT = np.ascontiguousarray(
            xblk.T.reshape(DS, P, T_HALO).transpose(1, 0, 2))
        kidx = np.arange(T_HALO, dtype=np.float32)
        if hi - q0 < T_HALO:
            kidx[hi - q0:] += 1e9
        kal = np.stack([kidx, np.ones(T_HALO, np.float32)])
        qal = np.empty((2, N_HEADS, T_OWN), np.float32)
        for h in range(N_HEADS):
            qal[0, h, :] = -8.0 * SLOPES[h]
            qal[1, h, :] = 8.0 * SLOPES[h] * np.arange(T_OWN)
        per_core.append({
            "xT": xT, "wq": wq_t, "wk": wk_t, "wv": wv_t, "wo": wo_t,
            "w1": w1_t, "w3": w3_t, "w2": w2_t, "qkw": qkw,
            "asc": asc, "fsc": fsc, "kal": kal, "qal": qal,
        })
    return per_core


_NC_CACHE = None
LAST_RESULT = None  # BassKernelResults of the most recent run (for profiling)
TRACE = False


def kernel(**inputs):
    global _NC_CACHE, LAST_RESULT
    per_core = _prep_inputs(**inputs)
    if _NC_CACHE is None:
        _NC_CACHE = _build_nc()
    res = run_bass_kernel_spmd(_NC_CACHE, per_core, core_ids=list(range(8)),
                               trace=TRACE)
    LAST_RESULT = res
    full = np.empty((B, S, DIM), np.float32)
    for c in range(8):
        b, blk = c // 4, c % 4
        y = res.results[c]["out"]                      # [p, ds, tok]
        full[b, blk * T_OWN:(blk + 1) * T_OWN] = (
            y.transpose(2, 1, 0).reshape(T_OWN, DIM))
    return full



# revision 21
# speedup vs baseline: 1.2778x; 1.1310x over previous
"""Trainium2 Bass kernel for nn_CodecTransformerLayer (sparse window attention
+ GQA + ALiBi + SwiGLU FFN), 8-core data-parallel with forward-halo recompute.

Sharding: batch(2) x seq-block(4) = 8 shards, one per core. Each core computes
its own 512 tokens end-to-end; attention needs K/V for the next 512 tokens
(window is forward-looking: dist = j - i in [0, 512]), which the core
recomputes from a 512-token halo of x instead of communicating.

Layout: feature-on-partition ("transposed") activations everywhere. All
weights and x are pre-transposed/pre-tiled on the host into the exact SBUF
layouts, so every DMA is contiguous. LayerNorm partition-dim reductions use
all-ones matmuls (gives the mean replicated across partitions for free).
ALiBi + band mask enter the score matmul as two extra contraction rows
(rank-2 decomposition of the in-band mask); out-of-band positions are zeroed
on the exp output with affine_select; invalid halo keys (last block of each
batch) get a +1e9 key-index so their logit is ~-1e9.

Matmul dtypes: bf16 for projections/FFN (error is scaled by 1e-5 residual
scales), float32r (reduced-mantissa fp32, full PE rate at N>=256) for
attention scores/AV and LN statistics. Residual path stays exact fp32.
"""

import math

import numpy as np
import ml_dtypes

import concourse.bass as bass
import concourse.mybir as mybir
import concourse.tile as tile
from concourse import bacc
from concourse.bass_utils import run_bass_kernel_spmd

P = 128
DIM = 1024
N_HEADS = 16
N_KV = 8
HD = 64
HIDDEN = 4096
WINDOW = 512
NORM_EPS = 1e-5
QK_EPS = 1e-6
B = 2
S = 2048
T_OWN = 512          # tokens owned per core
T_HALO = 1024        # own + forward halo
DS = DIM // P        # 8 d-subtiles
KS = DIM // P        # 8 hd-subtiles for wo contraction
HS = HIDDEN // P     # 32 hidden subtiles

F32 = mybir.dt.float32
F32R = mybir.dt.float32r
BF16 = mybir.dt.bfloat16
F8 = mybir.dt.float8e4
DR = mybir.MatmulPerfMode.DoubleRow
AF = mybir.ActivationFunctionType
OP = mybir.AluOpType

# fp8 weight scales (folded out via silu scale / host-side fsc fold)
S1 = 16.0
S3 = 16.0
S2 = 64.0


def _alibi_slopes(n):
    start = 2.0 ** (-(2.0 ** (-(math.log2(n) - 3))))
    return [start * start ** i for i in range(n)]


SLOPES = _alibi_slopes(N_HEADS)


# ---------------------------------------------------------------------------
# device kernel
# ---------------------------------------------------------------------------

def _build_nc():
    nc = bacc.Bacc("TRN2")

    ins = {}
    ins["xT"] = nc.dram_tensor("xT", [P, DS, T_HALO], F32, kind="ExternalInput")
    ins["wq"] = nc.dram_tensor("wq", [8, P, DS, 128], BF16, kind="ExternalInput")
    ins["wk"] = nc.dram_tensor("wk", [4, P, DS, 128], BF16, kind="ExternalInput")
    ins["wv"] = nc.dram_tensor("wv", [P, DS, 512], BF16, kind="ExternalInput")
    ins["wo"] = nc.dram_tensor("wo", [P, KS, DIM], BF16, kind="ExternalInput")
    ins["w1"] = nc.dram_tensor("w1", [HS, P, DS, 128], F8, kind="ExternalInput")
    ins["w3"] = nc.dram_tensor("w3", [HS, P, DS, 128], F8, kind="ExternalInput")
    ins["w2"] = nc.dram_tensor("w2", [DS, P, HS, 128], F8, kind="ExternalInput")
    # qnw*knw folded, head-local layout [64(pad128), head]
    ins["qkw"] = nc.dram_tensor("qkw", [P, N_HEADS], F32, kind="ExternalInput")
    ins["asc"] = nc.dram_tensor("asc", [P, DS], F32, kind="ExternalInput")
    ins["fsc"] = nc.dram_tensor("fsc", [P, DS], F32, kind="ExternalInput")
    ins["kal"] = nc.dram_tensor("kal", [2, T_HALO], F32R, kind="ExternalInput")
    ins["qal"] = nc.dram_tensor("qal", [2, N_HEADS, T_OWN], F32R, kind="ExternalInput")
    # band masks (0 / -1e9) for k-chunks 0,1,4,5 of each 256-query block
    ins["msk"] = nc.dram_tensor("msk", [P, 4, 256], F32, kind="ExternalInput")

    out = nc.dram_tensor("out", [P, DS, T_OWN], F32, kind="ExternalOutput")

    with tile.TileContext(nc) as tc:
        _emit(nc, tc, ins, out)
    nc.finalize()
    return nc


def _ln_coeffs(nc, pool, psm, pss, inv_n, eps_ap):
    """From sum/sumsq psums (replicated across partitions), produce
    a = rstd and b = mean * rstd, both [128, 512] f32 replicated."""
    m_ = pool.tile([P, 512], F32, tag="ln_m")
    nc.vector.tensor_scalar_mul(m_[:], psm[:], inv_n)
    v_ = pool.tile([P, 512], F32, tag="ln_v")
    nc.vector.tensor_scalar_mul(v_[:], pss[:], inv_n)
    mm_ = pool.tile([P, 512], F32, tag="ln_mm")
    nc.vector.tensor_tensor(mm_[:], m_[:], m_[:], OP.mult)
    nc.vector.tensor_tensor(v_[:], v_[:], mm_[:], OP.subtract)
    s_ = pool.tile([P, 512], F32, tag="ln_s")
    nc.scalar.activation(s_[:], v_[:], AF.Sqrt, bias=eps_ap)
    nc.vector.reciprocal(s_[:], s_[:])
    b_ = pool.tile([P, 512], F32, tag="ln_b")
    nc.vector.tensor_tensor(b_[:], m_[:], s_[:], OP.mult)
    return s_, b_


def _emit(nc, tc, ins, out):
    frees = []  # keep single-tile pool handles alive; release LIFO at end

    def tile_single(shape, dtype, name):
        t, f = tc.tile(shape, dtype, name=name)
        frees.append(f)
        return t

    xT, wq, wk, wv, wo = ins["xT"], ins["wq"], ins["wk"], ins["wv"], ins["wo"]
    w1, w3, w2 = ins["w1"], ins["w3"], ins["w2"]
    qkw, asc, fsc = ins["qkw"], ins["asc"], ins["fsc"]
    kal, qal = ins["kal"], ins["qal"]

    # --- constants (kept for the whole kernel) -----------------------------
    ones_f = tile_single([P, P], F32, name="ones_f")
    nc.vector.memset(ones_f[:], 1.0)
    ones128 = tile_single([P, P], F32R, name="ones128")
    nc.vector.tensor_copy(ones128[:], ones_f[:])
    ones1 = tile_single([1, HD], F32R, name="ones1")
    nc.vector.tensor_copy(ones1[:], ones_f[0:1, 0:HD])
    qkw_sb = tile_single([P, N_HEADS], F32, name="qkw_sb")
    nc.sync.dma_start(qkw_sb[:], qkw[:])
    asc_sb = tile_single([P, DS], F32, name="asc_sb")
    nc.sync.dma_start(asc_sb[:], asc[:])
    fsc_sb = tile_single([P, DS], F32, name="fsc_sb")
    nc.sync.dma_start(fsc_sb[:], fsc[:])
    eps_n = tile_single([P, 1], F32, name="eps_n")
    nc.vector.memset(eps_n[:], NORM_EPS)
    eps_qk = tile_single([P, 1], F32, name="eps_qk")
    nc.vector.memset(eps_qk[:], QK_EPS)
    msk_sb = tile_single([P, 4, 256], F32, name="msk_sb")
    nc.sync.dma_start(msk_sb[:], ins["msk"][:])

    xTo = tile_single([P, DS, T_OWN], F32, name="xTo")
    nc.sync.dma_start(xTo[:], xT[:, :, 0:T_OWN])
    aoT = tile_single([P, KS, T_OWN], BF16, name="aoT")
    x2T = tile_single([P, DS, T_OWN], F32, name="x2T")

    woc = tile_single([P, KS, DIM], BF16, name="woc")
    nc.sync.dma_start(woc[:], wo[:])

    NQ = 256

    hT, free_hT = tc.tile([P, DS, T_HALO], BF16, name="hT")

    # ======================================================================
    # Phase 1: attn LN over halo tokens -> hT (bf16)
    # (attn_norm_w is folded into wq/wk/wv on the host)
    # ======================================================================
    xTh, free_xTh = tc.tile([P, DS, T_OWN], F32, name="xTh")
    nc.sync.dma_start(xTh[:], xT[:, :, T_OWN:T_HALO])
    with tc.tile_pool(name="p1c", bufs=3) as p1c, \
         tc.tile_pool(name="p1s", bufs=1) as p1s, \
         tc.tile_pool(name="psA1", bufs=2, space="PSUM") as psA1:
        for tci, xsrc in ((0, xTo), (1, xTh)):
            psm = psA1.tile([P, 512], F32, tag="st_mean")
            pss = psA1.tile([P, 512], F32, tag="st_sq")
            for ds in range(DS):
                xr = p1c.tile([P, 512], F32R, tag="xr")
                nc.vector.tensor_copy(xr[:], xsrc[:, ds])
                nc.tensor.matmul(psm[:], ones128[:], xr[:],
                                 start=(ds == 0), stop=(ds == DS - 1))
            for ds in range(DS):
                xq = p1c.tile([P, 512], F32R, tag="xq")
                nc.scalar.activation(xq[:], xsrc[:, ds], AF.Square)
                nc.tensor.matmul(pss[:], ones128[:], xq[:],
                                 start=(ds == 0), stop=(ds == DS - 1))
            s_, b_ = _ln_coeffs(nc, p1s, psm, pss, 1.0 / DIM, eps_n[:])
            for ds in range(DS):
                t_ = p1c.tile([P, 512], F32, tag="t")
                nc.vector.tensor_tensor(t_[:], xsrc[:, ds], s_[:], OP.mult)
                nc.vector.tensor_tensor(
                    hT[:, ds, tci * 512:(tci + 1) * 512], t_[:], b_[:],
                    OP.subtract)
    free_xTh()

    # ======================================================================
    # Phase 2: q/k/v projections + q/k LN (in-place) -> qext, kext, vext
    # qext[h]: rows 0..63 = q_ln (head h), row 64 = -8*slope, row 65 =
    # 8*slope*qidx. kext[g]: rows 0..63 = k_ln, row 64 = kidx, row 65 = 1.
    # vext: [tok_p, tok_sub, kv*(HD+1)] with a ones column per kv head.
    # ======================================================================
    qext, free_qext = tc.tile([P, N_HEADS, T_OWN], F32R, name="qext")
    kext, free_kext = tc.tile([P, N_KV, T_HALO], F32R, name="kext")
    vext, free_vext = tc.tile([P, DS, N_KV * (HD + 1)], F32R, name="vext")

    with tc.tile_pool(name="p2w", bufs=3) as p2w, \
         tc.tile_pool(name="p2c", bufs=2) as p2c, \
         tc.tile_pool(name="p2s", bufs=1) as p2s, \
         tc.tile_pool(name="psA2", bufs=1, space="PSUM") as psA2, \
         tc.tile_pool(name="psA2p", bufs=2, space="PSUM") as psA2p:

        # ---- q projection + interleaved q-LN stats ----
        psm = psA2.tile([P, 512], F32, tag="st_mean")
        pss = psA2.tile([P, 512], F32, tag="st_sq")
        for fs in range(DS):
            wqc = p2w.tile([P, DS, 128], BF16, tag="wqc")
            nc.sync.dma_start(wqc[:], wq[fs])
            ps = psA2p.tile([P, 512], F32, tag="proj")
            for ds in range(DS):
                nc.tensor.matmul(ps[:], wqc[:, ds], hT[:, ds, 0:T_OWN],
                                 start=(ds == 0), stop=(ds == DS - 1))
            for half in range(2):
                h = fs * 2 + half
                nc.vector.tensor_copy(qext[0:HD, h, :],
                                      ps[half * HD:(half + 1) * HD, :])
                qsq = p2c.tile([P, 512], F32R, tag="qsq")
                nc.scalar.activation(qsq[0:HD, :], qext[0:HD, h, :], AF.Square)
                nc.tensor.matmul(psm[:], ones128[0:HD, :], qext[0:HD, h, :],
                                 start=(h == 0), stop=(h == N_HEADS - 1))
                nc.tensor.matmul(pss[:], ones128[0:HD, :], qsq[0:HD, :],
                                 start=(h == 0), stop=(h == N_HEADS - 1))
        s_, b_ = _ln_coeffs(nc, p2s, psm, pss, 1.0 / DIM, eps_qk[:])
        for h in range(N_HEADS):
            nc.vector.tensor_tensor(qext[0:HD, h, :], qext[0:HD, h, :],
                                    s_[0:HD, :], OP.mult)
            nc.vector.tensor_tensor(qext[0:HD, h, :], qext[0:HD, h, :],
                                    b_[0:HD, :], OP.subtract)
            nc.vector.tensor_scalar_mul(qext[0:HD, h, :], qext[0:HD, h, :],
                                        qkw_sb[0:HD, h:h + 1])
        nc.sync.dma_start(qext[HD:HD + 2, :, :], qal[:])

        # ---- k projection + interleaved k-LN stats (per token chunk) ----
        kstat = []
        for tci in range(2):
            kpsm = psA2.tile([P, 512], F32, tag=f"kst_mean{tci}")
            kpss = psA2.tile([P, 512], F32, tag=f"kst_sq{tci}")
            kstat.append((kpsm, kpss))
        for fs in range(4):
            wkc = p2w.tile([P, DS, 128], BF16, tag="wkc")
            nc.sync.dma_start(wkc[:], wk[fs])
            for tci in range(2):
                tsl = slice(tci * 512, (tci + 1) * 512)
                ps = psA2p.tile([P, 512], F32, tag="proj")
                for ds in range(DS):
                    nc.tensor.matmul(ps[:], wkc[:, ds], hT[:, ds, tsl],
                                     start=(ds == 0), stop=(ds == DS - 1))
                psm, pss = kstat[tci]
                for half in range(2):
                    g = fs * 2 + half
                    nc.vector.tensor_copy(kext[0:HD, g, tsl],
                                          ps[half * HD:(half + 1) * HD, :])
                    ksq = p2c.tile([P, 512], F32R, tag="ksq")
                    nc.scalar.activation(ksq[0:HD, :], kext[0:HD, g, tsl],
                                         AF.Square)
                    nc.tensor.matmul(psm[:], ones128[0:HD, :],
                                     kext[0:HD, g, tsl],
                                     start=(g == 0), stop=(g == N_KV - 1))
                    nc.tensor.matmul(pss[:], ones128[0:HD, :], ksq[0:HD, :],
                                     start=(g == 0), stop=(g == N_KV - 1))
        for tci in range(2):
            tsl = slice(tci * 512, (tci + 1) * 512)
            psm, pss = kstat[tci]
            s_, b_ = _ln_coeffs(nc, p2s, psm, pss, 1.0 / (N_KV * HD),
                                eps_qk[:])
            for g in range(N_KV):
                nc.vector.tensor_tensor(kext[0:HD, g, tsl], kext[0:HD, g, tsl],
                                        s_[0:HD, :], OP.mult)
                nc.vector.tensor_tensor(kext[0:HD, g, tsl], kext[0:HD, g, tsl],
                                        b_[0:HD, :], OP.subtract)
        for g in range(N_KV):
            nc.sync.dma_start(kext[HD:HD + 2, g, :], kal[:])

        # ---- v projection ----
        vv0 = vext[:].rearrange("p s (g e) -> p s g e", e=HD + 1)
        nc.vector.tensor_copy(
            vv0[:, :, :, HD:HD + 1],
            ones_f[:, 0:DS * N_KV].rearrange("p (s g) -> p s g", g=N_KV)[:, :, :, None])
        wvc, free_wvc = tc.tile([P, DS, 512], BF16, name="wvc")
        nc.sync.dma_start(wvc[:], wv[:])
        vview = vext[:].rearrange("p s (g e) -> p s g e", e=HD + 1)
        for ts8 in range(DS):
            ps = psA2p.tile([P, 512], F32, tag="proj")
            for ds in range(DS):
                nc.tensor.matmul(
                    ps[:], hT[:, ds, ts8 * 128:(ts8 + 1) * 128],
                    wvc[:, ds], start=(ds == 0), stop=(ds == DS - 1))
            nc.vector.tensor_copy(
                vview[:, ts8, :, 0:HD],
                ps[:].rearrange("p (g e) -> p g e", e=HD))
        free_wvc()

    # ======================================================================
    # Phase 3: attention units (16 heads x 2 q-blocks of 256)
    # ======================================================================
    NKC = 6
    with tc.tile_pool(name="p3", bufs=3) as p3, \
         tc.tile_pool(name="psB1", bufs=2, space="PSUM") as psB1, \
         tc.tile_pool(name="psB2", bufs=2, space="PSUM") as psB2:
        for h in range(N_HEADS):
            g = h // 2
            for t2 in range(2):
                sc = psB1.tile([P, NKC * NQ], F32, tag="sc")
                for kc in range(NKC):
                    ks = t2 * 2 + kc
                    nc.tensor.matmul(
                        sc[:, kc * NQ:(kc + 1) * NQ],
                        kext[0:HD + 2, g, ks * 128:(ks + 1) * 128],
                        qext[0:HD + 2, h, t2 * NQ:(t2 + 1) * NQ],
                        start=True, stop=True)
                # band mask: dist = kc*128 + r - j ; keep 0 <= dist <= 512
                # (additive -1e9 on psum before exp; kc 2,3 are fully in-band)
                for mi, kc in enumerate((0, 1, 4, 5)):
                    nc.vector.tensor_tensor(
                        sc[:, kc * NQ:(kc + 1) * NQ],
                        sc[:, kc * NQ:(kc + 1) * NQ],
                        msk_sb[:, mi], OP.add)
                expS = p3.tile([P, NKC * NQ], F32R, tag="expS")
                nc.scalar.activation(expS[:], sc[:], AF.Exp, scale=0.125)
                avdr = psB2.tile([HD + 1, 2 * NQ], F32, tag="avdr")
                av = avdr[:, 0:NQ]
                dr = avdr[0:HD, NQ:2 * NQ]
                vv = vext[:].rearrange("p s (g e) -> p s g e", e=HD + 1)
                for kc in range(NKC):
                    ks = t2 * 2 + kc
                    nc.tensor.matmul(
                        av[:], vv[:, ks, g, :],
                        expS[:, kc * NQ:(kc + 1) * NQ],
                        start=(kc == 0), stop=(kc == NKC - 1))
                dsb = p3.tile([1, NQ], F32R, tag="dsb")
                nc.scalar.copy(dsb[:], av[HD:HD + 1, :])
                nc.tensor.matmul(dr[:], ones1[:], dsb[:],
                                 start=True, stop=True)
                rsb = p3.tile([HD, NQ], F32, tag="rsb")
                nc.vector.reciprocal_approx_fast(rsb[:], dr)
                r0 = (h % 2) * HD
                nc.vector.tensor_tensor(
                    aoT[r0:r0 + HD, h // 2, t2 * NQ:(t2 + 1) * NQ],
                    av[0:HD, :], rsb[:], OP.mult)
    free_vext()
    free_kext()
    free_qext()
    free_hT()

    # ======================================================================
    # Phase 4: wo projection + residual -> x2T ; ffn LN -> h2T
    # ======================================================================
    h2T = tile_single([P, DS, T_OWN], F8, name="h2T")
    with tc.tile_pool(name="p4", bufs=2) as p4, \
         tc.tile_pool(name="p4s", bufs=1) as p4s, \
         tc.tile_pool(name="psC", bufs=2, space="PSUM") as psC:
        for ds2 in range(DS):
            ps = psC.tile([P, 512], F32, tag="proj")
            for hs8 in range(KS):
                nc.tensor.matmul(
                    ps[:], woc[:, hs8, ds2 * 128:(ds2 + 1) * 128],
                    aoT[:, hs8], start=(hs8 == 0), stop=(hs8 == KS - 1))
            t_ = p4.tile([P, 512], F32, tag="t")
            nc.vector.tensor_scalar_mul(t_[:], ps[:], asc_sb[:, ds2:ds2 + 1])
            nc.vector.tensor_tensor(x2T[:, ds2], t_[:], xTo[:, ds2], OP.add)

        # ffn LN (ffn_norm_w folded into w1/w3)
        psm = psC.tile([P, 512], F32, tag="st_mean")
        pss = psC.tile([P, 512], F32, tag="st_sq")
        for ds in range(DS):
            xr = p4.tile([P, 512], F32R, tag="xr")
            nc.vector.tensor_copy(xr[:], x2T[:, ds])
            nc.tensor.matmul(psm[:], ones128[:], xr[:],
                             start=(ds == 0), stop=(ds == DS - 1))
        for ds in range(DS):
            xq = p4.tile([P, 512], F32R, tag="xq")
            nc.scalar.activation(xq[:], x2T[:, ds], AF.Square)
            nc.tensor.matmul(pss[:], ones128[:], xq[:],
                             start=(ds == 0), stop=(ds == DS - 1))
        s_, b_ = _ln_coeffs(nc, p4s, psm, pss, 1.0 / DIM, eps_n[:])
        for ds in range(DS):
            t_ = p4.tile([P, 512], F32, tag="t")
            nc.vector.tensor_tensor(t_[:], x2T[:, ds], s_[:], OP.mult)
            nc.vector.tensor_tensor(h2T[:, ds], t_[:], b_[:], OP.subtract)

    # ======================================================================
    # Phase 5: SwiGLU FFN + residual -> out
    # ======================================================================
    gT, free_gT = tc.tile([P, HS, T_OWN], F8, name="gT")
    w2sb, free_w2sb = tc.tile([P, DS, HS, 128], F8, name="w2sb")
    for ds2 in range(DS):
        nc.scalar.dma_start(w2sb[:, ds2], w2[ds2])
    with tc.tile_pool(name="p5", bufs=3) as p5, \
         tc.tile_pool(name="p5w", bufs=8) as p5w, \
         tc.tile_pool(name="psD", bufs=1, space="PSUM") as psD, \
         tc.tile_pool(name="psDy", bufs=2, space="PSUM") as psDy:
        for hs2 in range(HS // 2):
            w1c = p5w.tile([P, DS, 256], F8, tag="w1c")
            w3c = p5w.tile([P, DS, 256], F8, tag="w3c")
            for half in range(2):
                fsl = slice(half * 128, (half + 1) * 128)
                nc.sync.dma_start(w1c[:, :, fsl], w1[hs2 * 2 + half])
                nc.sync.dma_start(w3c[:, :, fsl], w3[hs2 * 2 + half])
            psu = psD.tile([P, 1024], F32, tag="u")
            psw = psD.tile([P, 1024], F32, tag="w")
            for half in range(2):
                fsl = slice(half * 128, (half + 1) * 128)
                for sup in range(DS // 2):
                    nc.tensor.matmul(psu[:, half * 512:(half + 1) * 512],
                                     w1c[:, 2 * sup:2 * sup + 2, fsl],
                                     h2T[:, 2 * sup:2 * sup + 2, :],
                                     start=(sup == 0), stop=(sup == DS // 2 - 1),
                                     perf_mode=DR)
                for sup in range(DS // 2):
                    nc.tensor.matmul(psw[:, half * 512:(half + 1) * 512],
                                     w3c[:, 2 * sup:2 * sup + 2, fsl],
                                     h2T[:, 2 * sup:2 * sup + 2, :],
                                     start=(sup == 0), stop=(sup == DS // 2 - 1),
                                     perf_mode=DR)
            sil = p5.tile([P, 1024], F32, tag="sil")
            nc.scalar.activation(sil[:], psu[:], AF.Silu, scale=1.0 / S1)
            nc.vector.tensor_tensor(
                gT[:, hs2 * 2:hs2 * 2 + 2].rearrange("p a b -> p (a b)"),
                sil[:], psw[:], OP.mult)

        for ds2 in range(DS):
            psy = psDy.tile([P, 512], F32, tag="y")
            for sup in range(HS // 2):
                nc.tensor.matmul(psy[:], w2sb[:, ds2, 2 * sup:2 * sup + 2, :],
                                 gT[:, 2 * sup:2 * sup + 2, :],
                                 start=(sup == 0), stop=(sup == HS // 2 - 1),
                                 perf_mode=DR)
            t_ = p5.tile([P, 512], F32, tag="t")
            nc.vector.tensor_scalar_mul(t_[:], psy[:], fsc_sb[:, ds2:ds2 + 1])
            yv = p5.tile([P, 512], F32, tag="yv")
            nc.vector.tensor_tensor(yv[:], t_[:], x2T[:, ds2], OP.add)
            nc.sync.dma_start(out[:, ds2, :], yv[:])
    free_w2sb()
    free_gT()
    for f in reversed(frees):
        f()


# ---------------------------------------------------------------------------
# host side
# ---------------------------------------------------------------------------

def _tile_kxf(wT, f_chunk):
    """[K, F] (K=contraction, multiple of 128) -> [F//f_chunk, 128, K//128,
    f_chunk] chunks whose DMA into SBUF [p, ksub, f_chunk] is contiguous."""
    K, F = wT.shape
    return np.ascontiguousarray(
        wT.reshape(K // P, P, F // f_chunk, f_chunk).transpose(2, 1, 0, 3))


def _prep_inputs(x, wq, wk, wv, wo, q_norm_w, k_norm_w, attn_norm_w,
                 ffn_norm_w, w1, w2, w3, attn_scale, ffn_scale):
    bf = ml_dtypes.bfloat16
    x = np.asarray(x, np.float32)
    wq = np.asarray(wq, np.float32)
    wk = np.asarray(wk, np.float32)
    wv = np.asarray(wv, np.float32)
    wo = np.asarray(wo, np.float32)
    w1 = np.asarray(w1, np.float32)
    w2 = np.asarray(w2, np.float32)
    w3 = np.asarray(w3, np.float32)
    q_norm_w = np.asarray(q_norm_w, np.float32)
    k_norm_w = np.asarray(k_norm_w, np.float32)
    attn_norm_w = np.asarray(attn_norm_w, np.float32)
    ffn_norm_w = np.asarray(ffn_norm_w, np.float32)
    attn_scale = np.asarray(attn_scale, np.float32)
    ffn_scale = np.asarray(ffn_scale, np.float32)

    # fold attn_norm into wq/wk/wv, ffn_norm into w1/w3 (column scales)
    wq_e = wq * attn_norm_w[None, :]
    wk_e = wk * attn_norm_w[None, :]
    wv_e = wv * attn_norm_w[None, :]
    w1_e = w1 * ffn_norm_w[None, :]
    w3_e = w3 * ffn_norm_w[None, :]

    wq_t = _tile_kxf(wq_e.T, 128).astype(bf)           # [8,128,8,128]
    wk_t = _tile_kxf(wk_e.T, 128).astype(bf)           # [4,128,8,128]
    wv_t = np.ascontiguousarray(
        wv_e.T.reshape(DS, P, 512).transpose(1, 0, 2)).astype(bf)
    wo_t = np.ascontiguousarray(
        wo.T.reshape(KS, P, DIM).transpose(1, 0, 2)).astype(bf)
    f8 = ml_dtypes.float8_e4m3
    w1_t = _tile_kxf(w1_e.T * S1, 128).astype(f8)      # [32,128,8,128]
    w3_t = _tile_kxf(w3_e.T * S3, 128).astype(f8)
    w2_t = _tile_kxf(w2.T * S2, 128).astype(f8)        # [8,128,32,128]

    # qnw*knw folded, head-local layout [p(<64), h]
    qkw = np.zeros((P, N_HEADS), np.float32)
    for h in range(N_HEADS):
        qf = h * HD + np.arange(HD)
        kf = (h // 2) * HD + np.arange(HD)
        qkw[0:HD, h] = q_norm_w[qf] * k_norm_w[kf]

    def vec_tile(v):
        return np.ascontiguousarray(v.reshape(DS, P).T)

    asc = vec_tile(attn_scale)
    fsc = vec_tile(ffn_scale) / (S2 * S3)

    # band masks: dist = kc*128 + r - j ; keep 0 <= dist <= 512
    r = np.arange(P)[:, None]
    j = np.arange(256)[None, :]
    msk = np.zeros((P, 4, 256), np.float32)
    for mi, kc in enumerate((0, 1, 4, 5)):
        dist = kc * 128 + r - j
        msk[:, mi, :] = np.where((dist >= 0) & (dist <= WINDOW), 0.0, -1e9)

    per_core = []
    for c in range(8):
        b, blk = c // 4, c % 4
        q0 = blk * T_OWN
        hi = min(q0 + T_HALO, S)
        xblk = np.zeros((T_HALO, DIM), np.float32)
        xblk[0:hi - q0] = x[b, q0:hi]
        xT = np.ascontiguousarray(
            xblk.T.reshape(DS, P, T_HALO).transpose(1, 0, 2))
        kidx = np.arange(T_HALO, dtype=np.float32)
        if hi - q0 < T_HALO:
            kidx[hi - q0:] += 1e9
        kal = np.stack([kidx, np.ones(T_HALO, np.float32)])
        qal = np.empty((2, N_HEADS, T_OWN), np.float32)
        for h in range(N_HEADS):
            qal[0, h, :] = -8.0 * SLOPES[h]
            qal[1, h, :] = 8.0 * SLOPES[h] * np.arange(T_OWN)
        per_core.append({
            "xT": xT, "wq": wq_t, "wk": wk_t, "wv": wv_t, "wo": wo_t,
            "w1": w1_t, "w3": w3_t, "w2": w2_t, "qkw": qkw,
            "asc": asc, "fsc": fsc, "kal": kal, "qal": qal, "msk": msk,
        })
    return per_core


_NC_CACHE = None
LAST_RESULT = None  # BassKernelResults of the most recent run (for profiling)
TRACE = False


def kernel(**inputs):
    global _NC_CACHE, LAST_RESULT
    per_core = _prep_inputs(**inputs)
    if _NC_CACHE is None:
        _NC_CACHE = _build_nc()
    res = run_bass_kernel_spmd(_NC_CACHE, per_core, core_ids=list(range(8)),
                               trace=TRACE)
    LAST_RESULT = res
    full = np.empty((B, S, DIM), np.float32)
    for c in range(8):
        b, blk = c // 4, c % 4
        y = res.results[c]["out"]                      # [p, ds, tok]
        full[b, blk * T_OWN:(blk + 1) * T_OWN] = (
            y.transpose(2, 1, 0).reshape(T_OWN, DIM))
    return full



# revision 35
# speedup vs baseline: 1.3488x; 1.0556x over previous
"""Trainium2 Bass kernel for nn_CodecTransformerLayer (sparse window attention
+ GQA + ALiBi + SwiGLU FFN), 8-core data-parallel with forward-halo recompute.

Sharding: batch(2) x seq-block(4) = 8 shards, one per core. Each core computes
its own 512 tokens end-to-end; attention needs K/V for the next 512 tokens
(window is forward-looking: dist = j - i in [0, 512]), which the core
recomputes from a 512-token halo of x instead of communicating.

Layout: feature-on-partition ("transposed") activations everywhere. All
weights and x are pre-transposed/pre-tiled on the host into the exact SBUF
layouts, so every DMA is contiguous. LayerNorm partition-dim reductions use
all-ones matmuls (gives the mean replicated across partitions for free).
ALiBi + band mask enter the score matmul as two extra contraction rows
(rank-2 decomposition of the in-band mask); out-of-band positions are zeroed
on the exp output with affine_select; invalid halo keys (last block of each
batch) get a +1e9 key-index so their logit is ~-1e9.

Matmul dtypes: bf16 for projections/FFN (error is scaled by 1e-5 residual
scales), float32r (reduced-mantissa fp32, full PE rate at N>=256) for
attention scores/AV and LN statistics. Residual path stays exact fp32.
"""

import math

import numpy as np
import ml_dtypes

import concourse.bass as bass
import concourse.mybir as mybir
import concourse.tile as tile
from concourse import bacc
from concourse.bass_utils import run_bass_kernel_spmd

P = 128
DIM = 1024
N_HEADS = 16
N_KV = 8
HD = 64
HIDDEN = 4096
WINDOW = 512
NORM_EPS = 1e-5
QK_EPS = 1e-6
B = 2
S = 2048
T_OWN = 512          # tokens owned per core
T_HALO = 1024        # own + forward halo
DS = DIM // P        # 8 d-subtiles
KS = DIM // P        # 8 hd-subtiles for wo contraction
HS = HIDDEN // P     # 32 hidden subtiles

F32 = mybir.dt.float32
F32R = mybir.dt.float32r
BF16 = mybir.dt.bfloat16
F8 = mybir.dt.float8e4
DR = mybir.MatmulPerfMode.DoubleRow
AF = mybir.ActivationFunctionType
OP = mybir.AluOpType

# fp8 weight scales (folded out via silu scale / host-side fsc+asc folds;
# q/k scales cancel inside the q/k layernorms)
S1 = 16.0
S3 = 16.0
S2 = 64.0
SQK = 64.0
SV = 16.0
SO = 64.0


def _alibi_slopes(n):
    start = 2.0 ** (-(2.0 ** (-(math.log2(n) - 3))))
    return [start * start ** i for i in range(n)]


SLOPES = _alibi_slopes(N_HEADS)


# ---------------------------------------------------------------------------
# device kernel
# ---------------------------------------------------------------------------

def _build_nc():
    nc = bacc.Bacc("TRN2")

    ins = {}
    ins["xT"] = nc.dram_tensor("xT", [P, DS, T_HALO], F32R, kind="ExternalInput")
    ins["wq"] = nc.dram_tensor("wq", [8, P, DS, 128], F8, kind="ExternalInput")
    ins["wk"] = nc.dram_tensor("wk", [4, P, DS, 128], F8, kind="ExternalInput")
    ins["wv"] = nc.dram_tensor("wv", [P, DS, 512], F8, kind="ExternalInput")
    ins["wo"] = nc.dram_tensor("wo", [P, KS, DIM], F8, kind="ExternalInput")
    ins["w1"] = nc.dram_tensor("w1", [HS, P, DS, 128], F8, kind="ExternalInput")
    ins["w3"] = nc.dram_tensor("w3", [HS, P, DS, 128], F8, kind="ExternalInput")
    ins["w2"] = nc.dram_tensor("w2", [DS, P, HS, 128], F8, kind="ExternalInput")
    # qnw*knw folded, head-local layout [64(pad128), head]
    ins["qkw"] = nc.dram_tensor("qkw", [P, N_HEADS], F32, kind="ExternalInput")
    ins["asc"] = nc.dram_tensor("asc", [P, DS], F32, kind="ExternalInput")
    ins["fsc"] = nc.dram_tensor("fsc", [P, DS], F32, kind="ExternalInput")
    ins["kal"] = nc.dram_tensor("kal", [2, T_HALO], F32R, kind="ExternalInput")
    ins["qal"] = nc.dram_tensor("qal", [2, N_HEADS, T_OWN], F32R, kind="ExternalInput")
    # band masks (0 / -1e9) for k-chunks 0,1,4,5 of each 256-query block
    ins["msk"] = nc.dram_tensor("msk", [P, 4, 256], F32, kind="ExternalInput")

    out = nc.dram_tensor("out", [P, DS, T_OWN], F32, kind="ExternalOutput")

    with tile.TileContext(nc) as tc:
        _emit(nc, tc, ins, out)
    nc.finalize()
    return nc


def _ln_coeffs(nc, pool, psm, pss, inv_n, eps_ap):
    """From sum/sumsq psums (replicated across partitions), produce
    a = rstd and b = mean * rstd, both [128, 512] f32 replicated."""
    m_ = pool.tile([P, 512], F32, tag="ln_m")
    nc.vector.tensor_scalar_mul(m_[:], psm[:], inv_n)
    v_ = pool.tile([P, 512], F32, tag="ln_v")
    nc.vector.tensor_scalar_mul(v_[:], pss[:], inv_n)
    mm_ = pool.tile([P, 512], F32, tag="ln_mm")
    nc.vector.tensor_tensor(mm_[:], m_[:], m_[:], OP.mult)
    nc.vector.tensor_tensor(v_[:], v_[:], mm_[:], OP.subtract)
    s_ = pool.tile([P, 512], F32, tag="ln_s")
    nc.scalar.activation(s_[:], v_[:], AF.Sqrt, bias=eps_ap)
    nc.vector.reciprocal(s_[:], s_[:])
    b_ = pool.tile([P, 512], F32, tag="ln_b")
    nc.vector.tensor_tensor(b_[:], m_[:], s_[:], OP.mult)
    return s_, b_


def _emit(nc, tc, ins, out):
    frees = []  # keep single-tile pool handles alive; release LIFO at end

    def tile_single(shape, dtype, name):
        t, f = tc.tile(shape, dtype, name=name)
        frees.append(f)
        return t

    xT, wq, wk, wv, wo = ins["xT"], ins["wq"], ins["wk"], ins["wv"], ins["wo"]
    w1, w3, w2 = ins["w1"], ins["w3"], ins["w2"]
    qkw, asc, fsc = ins["qkw"], ins["asc"], ins["fsc"]
    kal, qal = ins["kal"], ins["qal"]

    # --- constants (kept for the whole kernel) -----------------------------
    ones_f = tile_single([P, P], F32, name="ones_f")
    nc.vector.memset(ones_f[:], 1.0)
    ones128 = tile_single([P, P], F32R, name="ones128")
    nc.vector.tensor_copy(ones128[:], ones_f[:])
    ones1 = tile_single([1, HD], BF16, name="ones1")
    nc.vector.tensor_copy(ones1[:], ones_f[0:1, 0:HD])
    qkw_sb = tile_single([P, N_HEADS], F32, name="qkw_sb")
    nc.sync.dma_start(qkw_sb[:], qkw[:])
    asc_sb = tile_single([P, DS], F32, name="asc_sb")
    nc.sync.dma_start(asc_sb[:], asc[:])
    fsc_sb = tile_single([P, DS], F32, name="fsc_sb")
    nc.sync.dma_start(fsc_sb[:], fsc[:])
    eps_n = tile_single([P, 1], F32, name="eps_n")
    nc.vector.memset(eps_n[:], NORM_EPS)
    eps_qk = tile_single([P, 1], F32, name="eps_qk")
    nc.vector.memset(eps_qk[:], QK_EPS)
    msk_sb = tile_single([P, 4, 256], F32, name="msk_sb")
    nc.sync.dma_start(msk_sb[:], ins["msk"][:])

    xTo = tile_single([P, DS, T_OWN], F32R, name="xTo")
    nc.sync.dma_start(xTo[:], xT[:, :, 0:T_OWN])
    aoT = tile_single([P, KS, T_OWN], F8, name="aoT")
    x2T = tile_single([P, DS, T_OWN], F32R, name="x2T")

    woc = tile_single([P, KS, DIM], F8, name="woc")
    nc.sync.dma_start(woc[:], wo[:])

    NQ = 256

    hT, free_hT = tc.tile([P, DS, T_HALO], F8, name="hT")

    # ======================================================================
    # Phase 1: attn LN over halo tokens -> hT (fp8)
    # (attn_norm_w is folded into wq/wk/wv on the host)
    # ======================================================================
    xTh, free_xTh = tc.tile([P, DS, T_OWN], F32R, name="xTh")
    nc.sync.dma_start(xTh[:], xT[:, :, T_OWN:T_HALO])
    with tc.tile_pool(name="p1c", bufs=3) as p1c, \
         tc.tile_pool(name="p1s", bufs=1) as p1s, \
         tc.tile_pool(name="psA1", bufs=2, space="PSUM") as psA1:
        for tci, xsrc in ((0, xTo), (1, xTh)):
            psm = psA1.tile([P, 512], F32, tag="st_mean")
            pss = psA1.tile([P, 512], F32, tag="st_sq")
            for ds in range(DS):
                nc.tensor.matmul(psm[:], ones128[:], xsrc[:, ds],
                                 start=(ds == 0), stop=(ds == DS - 1))
            for ds in range(DS):
                xq = p1c.tile([P, 512], F32R, tag="xq")
                nc.scalar.activation(xq[:], xsrc[:, ds], AF.Square)
                nc.tensor.matmul(pss[:], ones128[:], xq[:],
                                 start=(ds == 0), stop=(ds == DS - 1))
            s_, b_ = _ln_coeffs(nc, p1s, psm, pss, 1.0 / DIM, eps_n[:])
            for ds in range(DS):
                t_ = p1c.tile([P, 512], F32, tag="t")
                nc.vector.tensor_tensor(t_[:], xsrc[:, ds], s_[:], OP.mult)
                nc.vector.tensor_tensor(
                    hT[:, ds, tci * 512:(tci + 1) * 512], t_[:], b_[:],
                    OP.subtract)
    free_xTh()

    # ======================================================================
    # Phase 2: q/k/v projections + q/k LN (in-place) -> qext, kext, vext
    # qext[h]: rows 0..63 = q_ln (head h), row 64 = -8*slope, row 65 =
    # 8*slope*qidx. kext[g]: rows 0..63 = k_ln, row 64 = kidx, row 65 = 1.
    # vext: [tok_p, tok_sub, kv*(HD+1)] with a ones column per kv head.
    # ======================================================================
    qext, free_qext = tc.tile([P, N_HEADS, T_OWN], F32R, name="qext")
    kext, free_kext = tc.tile([P, N_KV, T_HALO], F32R, name="kext")
    vext, free_vext = tc.tile([P, DS, N_KV * (HD + 1)], F32R, name="vext")

    with tc.tile_pool(name="p2w", bufs=3) as p2w, \
         tc.tile_pool(name="p2c", bufs=2) as p2c, \
         tc.tile_pool(name="p2s", bufs=1) as p2s, \
         tc.tile_pool(name="psA2", bufs=1, space="PSUM") as psA2, \
         tc.tile_pool(name="psA2p", bufs=2, space="PSUM") as psA2p:

        # ---- q projection + interleaved q-LN stats ----
        psm = psA2.tile([P, 512], F32, tag="st_mean")
        pss = psA2.tile([P, 512], F32, tag="st_sq")
        for fs in range(DS):
            wqc = p2w.tile([P, DS, 128], F8, tag="wqc")
            nc.sync.dma_start(wqc[:], wq[fs])
            ps = psA2p.tile([P, 512], F32, tag="proj")
            for sup in range(DS // 2):
                nc.tensor.matmul(ps[:], wqc[:, 2 * sup:2 * sup + 2, :],
                                 hT[:, 2 * sup:2 * sup + 2, 0:T_OWN],
                                 start=(sup == 0), stop=(sup == DS // 2 - 1),
                                 perf_mode=DR)
            for half in range(2):
                h = fs * 2 + half
                nc.vector.tensor_copy(qext[0:HD, h, :],
                                      ps[half * HD:(half + 1) * HD, :])
                qsq = p2c.tile([P, 512], F32R, tag="qsq")
                nc.scalar.activation(qsq[0:HD, :], qext[0:HD, h, :], AF.Square)
                nc.tensor.matmul(psm[:], ones128[0:HD, :], qext[0:HD, h, :],
                                 start=(h == 0), stop=(h == N_HEADS - 1))
                nc.tensor.matmul(pss[:], ones128[0:HD, :], qsq[0:HD, :],
                                 start=(h == 0), stop=(h == N_HEADS - 1))
        s_, b_ = _ln_coeffs(nc, p2s, psm, pss, 1.0 / DIM, eps_qk[:])
        for h in range(N_HEADS):
            nc.vector.tensor_tensor(qext[0:HD, h, :], qext[0:HD, h, :],
                                    s_[0:HD, :], OP.mult)
            nc.vector.tensor_tensor(qext[0:HD, h, :], qext[0:HD, h, :],
                                    b_[0:HD, :], OP.subtract)
            nc.scalar.activation(qext[0:HD, h, :], qext[0:HD, h, :],
                                 AF.Copy, scale=qkw_sb[0:HD, h:h + 1])
        nc.sync.dma_start(qext[HD:HD + 2, :, :], qal[:])

        # ---- k projection + interleaved k-LN stats (per token chunk) ----
        kstat = []
        for tci in range(2):
            kpsm = psA2.tile([P, 512], F32, tag=f"kst_mean{tci}")
            kpss = psA2.tile([P, 512], F32, tag=f"kst_sq{tci}")
            kstat.append((kpsm, kpss))
        for fs in range(4):
            wkc = p2w.tile([P, DS, 128], F8, tag="wkc")
            nc.sync.dma_start(wkc[:], wk[fs])
            for tci in range(2):
                tsl = slice(tci * 512, (tci + 1) * 512)
                ps = psA2p.tile([P, 512], F32, tag="proj")
                for sup in range(DS // 2):
                    nc.tensor.matmul(ps[:], wkc[:, 2 * sup:2 * sup + 2, :],
                                     hT[:, 2 * sup:2 * sup + 2, tsl],
                                     start=(sup == 0),
                                     stop=(sup == DS // 2 - 1), perf_mode=DR)
                psm, pss = kstat[tci]
                for half in range(2):
                    g = fs * 2 + half
                    nc.vector.tensor_copy(kext[0:HD, g, tsl],
                                          ps[half * HD:(half + 1) * HD, :])
                    ksq = p2c.tile([P, 512], F32R, tag="ksq")
                    nc.scalar.activation(ksq[0:HD, :], kext[0:HD, g, tsl],
                                         AF.Square)
                    nc.tensor.matmul(psm[:], ones128[0:HD, :],
                                     kext[0:HD, g, tsl],
                                     start=(g == 0), stop=(g == N_KV - 1))
                    nc.tensor.matmul(pss[:], ones128[0:HD, :], ksq[0:HD, :],
                                     start=(g == 0), stop=(g == N_KV - 1))
        for tci in range(2):
            tsl = slice(tci * 512, (tci + 1) * 512)
            psm, pss = kstat[tci]
            s_, b_ = _ln_coeffs(nc, p2s, psm, pss, 1.0 / (N_KV * HD),
                                eps_qk[:])
            for g in range(N_KV):
                nc.vector.tensor_tensor(kext[0:HD, g, tsl], kext[0:HD, g, tsl],
                                        s_[0:HD, :], OP.mult)
                nc.vector.tensor_tensor(kext[0:HD, g, tsl], kext[0:HD, g, tsl],
                                        b_[0:HD, :], OP.subtract)
        for g in range(N_KV):
            nc.sync.dma_start(kext[HD:HD + 2, g, :], kal[:])

        # ---- v projection ----
        vv0 = vext[:].rearrange("p s (g e) -> p s g e", e=HD + 1)
        nc.vector.tensor_copy(
            vv0[:, :, :, HD:HD + 1],
            ones_f[:, 0:DS * N_KV].rearrange("p (s g) -> p s g", g=N_KV)[:, :, :, None])
        wvc, free_wvc = tc.tile([P, DS, 512], F8, name="wvc")
        nc.sync.dma_start(wvc[:], wv[:])
        vview = vext[:].rearrange("p s (g e) -> p s g e", e=HD + 1)
        for ts8 in range(DS):
            ps = psA2p.tile([P, 512], F32, tag="proj")
            for sup in range(DS // 2):
                nc.tensor.matmul(
                    ps[:], hT[:, 2 * sup:2 * sup + 2, ts8 * 128:(ts8 + 1) * 128],
                    wvc[:, 2 * sup:2 * sup + 2, :],
                    start=(sup == 0), stop=(sup == DS // 2 - 1), perf_mode=DR)
            nc.scalar.copy(
                vview[:, ts8, :, 0:HD],
                ps[:].rearrange("p (g e) -> p g e", e=HD))
        free_wvc()

    # ======================================================================
    # Phase 3: attention units (16 heads x 2 q-blocks of 256)
    # ======================================================================
    NKC = 6
    with tc.tile_pool(name="p3", bufs=3) as p3, \
         tc.tile_pool(name="psB1", bufs=2, space="PSUM") as psB1, \
         tc.tile_pool(name="psB2", bufs=2, space="PSUM") as psB2:
        for h in range(N_HEADS):
            g = h // 2
            for t2 in range(2):
                sc = psB1.tile([P, NKC * NQ], F32, tag="sc")
                for kc in range(NKC):
                    ks = t2 * 2 + kc
                    nc.tensor.matmul(
                        sc[:, kc * NQ:(kc + 1) * NQ],
                        kext[0:HD + 2, g, ks * 128:(ks + 1) * 128],
                        qext[0:HD + 2, h, t2 * NQ:(t2 + 1) * NQ],
                        start=True, stop=True)
                # band mask: dist = kc*128 + r - j ; keep 0 <= dist <= 512
                # (additive -1e9 on psum before exp; kc 2,3 are fully in-band;
                # kc=1 violations only at j>=128, kc=4 only at j<128)
                for mi, kc, j0, j1 in ((0, 0, 0, 256), (1, 1, 128, 256),
                                       (2, 4, 0, 128), (3, 5, 0, 256)):
                    nc.vector.tensor_tensor(
                        sc[:, kc * NQ + j0:kc * NQ + j1],
                        sc[:, kc * NQ + j0:kc * NQ + j1],
                        msk_sb[:, mi, j0:j1], OP.add)
                expS = p3.tile([P, NKC * NQ], F32R, tag="expS")
                nc.scalar.activation(expS[:], sc[:], AF.Exp, scale=0.125)
                avdr = psB2.tile([HD + 1, 2 * NQ], F32, tag="avdr")
                av = avdr[:, 0:NQ]
                dr = avdr[0:HD, NQ:2 * NQ]
                vv = vext[:].rearrange("p s (g e) -> p s g e", e=HD + 1)
                for kc in range(NKC):
                    ks = t2 * 2 + kc
                    nc.tensor.matmul(
                        av[:], vv[:, ks, g, :],
                        expS[:, kc * NQ:(kc + 1) * NQ],
                        start=(kc == 0), stop=(kc == NKC - 1))
                dsb = p3.tile([1, NQ], BF16, tag="dsb")
                nc.scalar.copy(dsb[:], av[HD:HD + 1, :])
                nc.tensor.matmul(dr[:], ones1[:], dsb[:],
                                 start=True, stop=True)
                rsb = p3.tile([HD, NQ], F32, tag="rsb")
                nc.vector.reciprocal_approx_fast(rsb[:], dr)
                r0 = (h % 2) * HD
                nc.vector.tensor_tensor(
                    aoT[r0:r0 + HD, h // 2, t2 * NQ:(t2 + 1) * NQ],
                    av[0:HD, :], rsb[:], OP.mult)
    free_vext()
    free_kext()
    free_qext()
    free_hT()

    # ======================================================================
    # Phase 4: wo projection + residual -> x2T ; ffn LN -> h2T
    # ======================================================================
    h2T = tile_single([P, DS, T_OWN], F8, name="h2T")
    with tc.tile_pool(name="p4", bufs=2) as p4, \
         tc.tile_pool(name="p4s", bufs=1) as p4s, \
         tc.tile_pool(name="psC", bufs=2, space="PSUM") as psC:
        for ds2 in range(DS):
            ps = psC.tile([P, 512], F32, tag="proj")
            for sup in range(KS // 2):
                nc.tensor.matmul(
                    ps[:], woc[:, 2 * sup:2 * sup + 2, ds2 * 128:(ds2 + 1) * 128],
                    aoT[:, 2 * sup:2 * sup + 2, :],
                    start=(sup == 0), stop=(sup == KS // 2 - 1), perf_mode=DR)
            t_ = p4.tile([P, 512], F32, tag="t")
            nc.vector.tensor_scalar_mul(t_[:], ps[:], asc_sb[:, ds2:ds2 + 1])
            nc.vector.tensor_tensor(x2T[:, ds2], t_[:], xTo[:, ds2], OP.add)

        # ffn LN (ffn_norm_w folded into w1/w3)
        psm = psC.tile([P, 512], F32, tag="st_mean")
        pss = psC.tile([P, 512], F32, tag="st_sq")
        for ds in range(DS):
            nc.tensor.matmul(psm[:], ones128[:], x2T[:, ds],
                             start=(ds == 0), stop=(ds == DS - 1))
        for ds in range(DS):
            xq = p4.tile([P, 512], F32R, tag="xq")
            nc.scalar.activation(xq[:], x2T[:, ds], AF.Square)
            nc.tensor.matmul(pss[:], ones128[:], xq[:],
                             start=(ds == 0), stop=(ds == DS - 1))
        s_, b_ = _ln_coeffs(nc, p4s, psm, pss, 1.0 / DIM, eps_n[:])
        for ds in range(DS):
            t_ = p4.tile([P, 512], F32, tag="t")
            nc.vector.tensor_tensor(t_[:], x2T[:, ds], s_[:], OP.mult)
            nc.vector.tensor_tensor(h2T[:, ds], t_[:], b_[:], OP.subtract)

    # ======================================================================
    # Phase 5: SwiGLU FFN + residual -> out
    # ======================================================================
    gT, free_gT = tc.tile([P, HS, T_OWN], F8, name="gT")
    w2sb, free_w2sb = tc.tile([P, DS, HS, 128], F8, name="w2sb")
    for ds2 in range(DS):
        nc.scalar.dma_start(w2sb[:, ds2], w2[ds2])
    with tc.tile_pool(name="p5", bufs=3) as p5, \
         tc.tile_pool(name="p5w", bufs=8) as p5w, \
         tc.tile_pool(name="psD", bufs=1, space="PSUM") as psD, \
         tc.tile_pool(name="psDy", bufs=2, space="PSUM") as psDy:
        for hs2 in range(HS // 2):
            w1c = p5w.tile([P, DS, 256], F8, tag="w1c")
            w3c = p5w.tile([P, DS, 256], F8, tag="w3c")
            for half in range(2):
                fsl = slice(half * 128, (half + 1) * 128)
                nc.sync.dma_start(w1c[:, :, fsl], w1[hs2 * 2 + half])
                nc.sync.dma_start(w3c[:, :, fsl], w3[hs2 * 2 + half])
            psu = psD.tile([P, 1024], F32, tag="u")
            psw = psD.tile([P, 1024], F32, tag="w")
            for half in range(2):
                fsl = slice(half * 128, (half + 1) * 128)
                for sup in range(DS // 2):
                    nc.tensor.matmul(psu[:, half * 512:(half + 1) * 512],
                                     w1c[:, 2 * sup:2 * sup + 2, fsl],
                                     h2T[:, 2 * sup:2 * sup + 2, :],
                                     start=(sup == 0), stop=(sup == DS // 2 - 1),
                                     perf_mode=DR)
                for sup in range(DS // 2):
                    nc.tensor.matmul(psw[:, half * 512:(half + 1) * 512],
                                     w3c[:, 2 * sup:2 * sup + 2, fsl],
                                     h2T[:, 2 * sup:2 * sup + 2, :],
                                     start=(sup == 0), stop=(sup == DS // 2 - 1),
                                     perf_mode=DR)
            sil = p5.tile([P, 1024], F32, tag="sil")
            nc.scalar.activation(sil[:], psu[:], AF.Silu, scale=1.0 / S1)
            nc.vector.tensor_tensor(
                gT[:, hs2 * 2:hs2 * 2 + 2].rearrange("p a b -> p (a b)"),
                sil[:], psw[:], OP.mult)

        for ds2 in range(DS):
            psy = psDy.tile([P, 512], F32, tag="y")
            for sup in range(HS // 2):
                nc.tensor.matmul(psy[:], w2sb[:, ds2, 2 * sup:2 * sup + 2, :],
                                 gT[:, 2 * sup:2 * sup + 2, :],
                                 start=(sup == 0), stop=(sup == HS // 2 - 1),
                                 perf_mode=DR)
            t_ = p5.tile([P, 512], F32, tag="t")
            nc.vector.tensor_scalar_mul(t_[:], psy[:], fsc_sb[:, ds2:ds2 + 1])
            yv = p5.tile([P, 512], F32, tag="yv")
            nc.vector.tensor_tensor(yv[:], t_[:], x2T[:, ds2], OP.add)
            nc.sync.dma_start(out[:, ds2, :], yv[:])
    free_w2sb()
    free_gT()
    for f in reversed(frees):
        f()


# ---------------------------------------------------------------------------
# host side
# ---------------------------------------------------------------------------

def _tile_kxf(wT, f_chunk):
    """[K, F] (K=contraction, multiple of 128) -> [F//f_chunk, 128, K//128,
    f_chunk] chunks whose DMA into SBUF [p, ksub, f_chunk] is contiguous."""
    K, F = wT.shape
    return np.ascontiguousarray(
        wT.reshape(K // P, P, F // f_chunk, f_chunk).transpose(2, 1, 0, 3))


def _prep_inputs(x, wq, wk, wv, wo, q_norm_w, k_norm_w, attn_norm_w,
                 ffn_norm_w, w1, w2, w3, attn_scale, ffn_scale):
    bf = ml_dtypes.bfloat16
    x = np.asarray(x, np.float32)
    wq = np.asarray(wq, np.float32)
    wk = np.asarray(wk, np.float32)
    wv = np.asarray(wv, np.float32)
    wo = np.asarray(wo, np.float32)
    w1 = np.asarray(w1, np.float32)
    w2 = np.asarray(w2, np.float32)
    w3 = np.asarray(w3, np.float32)
    q_norm_w = np.asarray(q_norm_w, np.float32)
    k_norm_w = np.asarray(k_norm_w, np.float32)
    attn_norm_w = np.asarray(attn_norm_w, np.float32)
    ffn_norm_w = np.asarray(ffn_norm_w, np.float32)
    attn_scale = np.asarray(attn_scale, np.float32)
    ffn_scale = np.asarray(ffn_scale, np.float32)

    # fold attn_norm into wq/wk/wv, ffn_norm into w1/w3 (column scales)
    wq_e = wq * attn_norm_w[None, :]
    wk_e = wk * attn_norm_w[None, :]
    wv_e = wv * attn_norm_w[None, :]
    w1_e = w1 * ffn_norm_w[None, :]
    w3_e = w3 * ffn_norm_w[None, :]

    f8 = ml_dtypes.float8_e4m3
    wq_t = _tile_kxf(wq_e.T * SQK, 128).astype(f8)     # [8,128,8,128]
    wk_t = _tile_kxf(wk_e.T * SQK, 128).astype(f8)     # [4,128,8,128]
    wv_t = np.ascontiguousarray(
        (wv_e.T * SV).reshape(DS, P, 512).transpose(1, 0, 2)).astype(f8)
    wo_t = np.ascontiguousarray(
        (wo.T * SO).reshape(KS, P, DIM).transpose(1, 0, 2)).astype(f8)
    w1_t = _tile_kxf(w1_e.T * S1, 128).astype(f8)      # [32,128,8,128]
    w3_t = _tile_kxf(w3_e.T * S3, 128).astype(f8)
    w2_t = _tile_kxf(w2.T * S2, 128).astype(f8)        # [8,128,32,128]

    # qnw*knw folded, head-local layout [p(<64), h]
    qkw = np.zeros((P, N_HEADS), np.float32)
    for h in range(N_HEADS):
        qf = h * HD + np.arange(HD)
        kf = (h // 2) * HD + np.arange(HD)
        qkw[0:HD, h] = q_norm_w[qf] * k_norm_w[kf]

    def vec_tile(v):
        return np.ascontiguousarray(v.reshape(DS, P).T)

    asc = vec_tile(attn_scale) / (SV * SO)
    fsc = vec_tile(ffn_scale) / (S2 * S3)

    # band masks: dist = kc*128 + r - j ; keep 0 <= dist <= 512
    r = np.arange(P)[:, None]
    j = np.arange(256)[None, :]
    msk = np.zeros((P, 4, 256), np.float32)
    for mi, kc in enumerate((0, 1, 4, 5)):
        dist = kc * 128 + r - j
        msk[:, mi, :] = np.where((dist >= 0) & (dist <= WINDOW), 0.0, -1e9)

    per_core = []
    for c in range(8):
        b, blk = c // 4, c % 4
        q0 = blk * T_OWN
        hi = min(q0 + T_HALO, S)
        xblk = np.zeros((T_HALO, DIM), np.float32)
        xblk[0:hi - q0] = x[b, q0:hi]
        xT = np.ascontiguousarray(
            xblk.T.reshape(DS, P, T_HALO).transpose(1, 0, 2))
        kidx = np.arange(T_HALO, dtype=np.float32)
        if hi - q0 < T_HALO:
            kidx[hi - q0:] += 1e9
        kal = np.stack([kidx, np.ones(T_HALO, np.float32)])
        qal = np.empty((2, N_HEADS, T_OWN), np.float32)
        for h in range(N_HEADS):
            qal[0, h, :] = -8.0 * SLOPES[h]
            qal[1, h, :] = 8.0 * SLOPES[h] * np.arange(T_OWN)
        per_core.append({
            "xT": xT, "wq": wq_t, "wk": wk_t, "wv": wv_t, "wo": wo_t,
            "w1": w1_t, "w3": w3_t, "w2": w2_t, "qkw": qkw,
            "asc": asc, "fsc": fsc, "kal": kal, "qal": qal, "msk": msk,
        })
    return per_core


_NC_CACHE = None
LAST_RESULT = None  # BassKernelResults of the most recent run (for profiling)
TRACE = False


def kernel(**inputs):
    global _NC_CACHE, LAST_RESULT
    per_core = _prep_inputs(**inputs)
    if _NC_CACHE is None:
        _NC_CACHE = _build_nc()
    res = run_bass_kernel_spmd(_NC_CACHE, per_core, core_ids=list(range(8)),
                               trace=TRACE)
    LAST_RESULT = res
    full = np.empty((B, S, DIM), np.float32)
    for c in range(8):
        b, blk = c // 4, c % 4
        y = res.results[c]["out"]                      # [p, ds, tok]
        full[b, blk * T_OWN:(blk + 1) * T_OWN] = (
            y.transpose(2, 1, 0).reshape(T_OWN, DIM))
    return full

